# revision 1
# baseline (speedup 1.0000x reference)
"""Trainium2 Bass kernel for nn_DecodeBlock (RetNet-style decoder block).

Sharding: data-parallel over batch (B=8) across the 8 NeuronCores; each core
computes the full block for one batch element. No collectives.

Algorithm notes (per core, feature-major "transposed" dataflow):
  - All activations are kept feature-major: X^T [E=512(4 part-tiles), S=1024].
  - Retention decay D[h,n,m] = kappa_h^(n-m) (causal) is applied via global
    row/col scaling: qs^T = q^T * kappa^n, ks^T = k^T * kappa^-m, then a 0/1
    causal mask on diagonal blocks only (exact in fp32: kappa^-1023 <= 1.3e14).
  - scoresT[m,n] tiles come straight from PE with m on partitions; ret^T is
    accumulated per head with V (seq-major) as the stationary operand.
  - GroupNorm/RMSNorm stats are computed with ones-matmuls over partitions
    (PE) and broadcast back with gpsimd partition_broadcast.
  - Matmuls run as float32r (TF32-like) at free-dim 512 -> 1 cycle/row.
"""

import numpy as np

import concourse.bass as bass
import concourse.mybir as mybir
import concourse.tile as tile
from concourse.bass_utils import run_bass_kernel_spmd

F32 = mybir.dt.float32
BF16 = mybir.dt.bfloat16
AF = mybir.ActivationFunctionType

E, H, B, S = 512, 8, 8, 1024
DH = E // H          # 64
P = 128
NF = E // P          # 4 feature tiles
NS = S // P          # 8 seq tiles
NH2 = S // 512       # 2 n-halves

N_CORES = 8


def _kappas():
    k = 1.0 - np.exp(np.linspace(np.log(1.0 / 32.0), np.log(1.0 / 512.0), H))
    return k.astype(np.float64)


def r(ap):
    return ap


def _build_consts(inputs):
    """Host-side constant tensors shared by all cores."""
    import ml_dtypes
    bf16 = ml_dtypes.bfloat16
    kap = _kappas()
    n = np.arange(S, dtype=np.float64)
    kq = np.empty((E, S), np.float64)
    kk = np.empty((E, S), np.float64)
    for h in range(H):
        kq[h * DH:(h + 1) * DH, :] = (kap[h] ** n)[None, :]
        kk[h * DH:(h + 1) * DH, :] = (kap[h] ** (-n))[None, :]
    kqmap = np.ascontiguousarray(kq.astype(bf16))
    kkmap = np.ascontiguousarray(kk.astype(bf16))
    # causal mask for a [128, 4*512] psum group: section i covers m-block
    # offset 128*i vs n-block base: keep if j >= p + 128*i
    cmask = np.zeros((P, 4 * 512), np.float32)  # cast to bf16 below
    jj = np.arange(512)
    for i in range(4):
        cmask[:, i * 512:(i + 1) * 512] = (jj[None, :] >= (np.arange(P)[:, None] + 128 * i))
    cmask = cmask.astype(bf16)
    gn_ones = np.zeros((P, 2), bf16)
    gn_ones[:64, 0] = 1.0 / DH
    gn_ones[64:, 1] = 1.0 / DH
    gn_bcast = np.zeros((2, P), bf16)
    gn_bcast[0, :64] = 1.0
    gn_bcast[1, 64:] = 1.0
    rms_ones = np.zeros((P, 2), bf16)
    rms_ones[:, 0] = 1.0
    rms_bcast = np.zeros((2, P), bf16)
    rms_bcast[0, :] = 1.0
    ident = np.eye(P, dtype=np.float32)

    def pp(v):  # [512] -> [128, 4] per-partition layout
        return np.ascontiguousarray(np.asarray(v, np.float32).reshape(NF, P).T)

    consts = {
        "kqmap": kqmap, "kkmap": kkmap, "cmask": cmask,
        "gn_ones": gn_ones, "gn_bcast": gn_bcast, "rms_ones": rms_ones,
        "rms_bcast": rms_bcast, "ident": ident,
        "gs1_pp": pp(inputs["gs1"]), "gb1_pp": pp(inputs["gb1"]),
        "gs2_pp": pp(inputs["gs2"]), "gb2_pp": pp(inputs["gb2"]),
        "ln1_pp": pp(inputs["ln1_s"]), "ln2_pp": pp(inputs["ln2_s"]),
        "ln3_pp": pp(inputs["ln3_s"]),
        "rl1": np.vstack([np.asarray(inputs["ln1_s"], np.float32),
                          np.zeros(E, np.float32)]).astype(bf16),
        "rl2": np.vstack([np.asarray(inputs["ln2_s"], np.float32),
                          np.zeros(E, np.float32)]).astype(bf16),
        "rl3": np.vstack([np.asarray(inputs["ln3_s"], np.float32),
                          np.zeros(E, np.float32)]).astype(bf16),
    }
    for nm in ("wq", "wk", "wv"):
        for i in (1, 2):
            w = np.asarray(inputs[f"{nm}{i}"], np.float32)      # [H, E, DH]
            consts[f"{nm}c{i}"] = np.ascontiguousarray(
                w.transpose(1, 0, 2).reshape(E, E).astype(bf16))
    for nm in ("wg1", "wo1", "wg2", "wo2", "ffn_w_gate", "ffn_w_lin", "ffn_w_out"):
        consts[nm] = np.ascontiguousarray(np.asarray(inputs[nm], np.float32).astype(bf16))
    return consts


class _Prog:
    pass


def _build_program():
    nc = bass.Bass()
    pr = _Prog()
    pr.nc = nc
    d = {}
    d["x"] = nc.dram_tensor("x", [S, E], F32, kind="ExternalInput")
    d["obs"] = nc.dram_tensor("obs", [S, E], F32, kind="ExternalInput")
    for nm in ("wqc1", "wkc1", "wvc1", "wqc2", "wkc2", "wvc2",
               "wg1", "wo1", "wg2", "wo2",
               "ffn_w_gate", "ffn_w_lin", "ffn_w_out"):
        d[nm] = nc.dram_tensor(nm, [E, E], BF16, kind="ExternalInput")
    d["cmask"] = nc.dram_tensor("cmask", [P, 4 * 512], BF16, kind="ExternalInput")
    d["gn_ones"] = nc.dram_tensor("gn_ones", [P, 2], BF16, kind="ExternalInput")
    d["gn_bcast"] = nc.dram_tensor("gn_bcast", [2, P], BF16, kind="ExternalInput")
    d["rms_ones"] = nc.dram_tensor("rms_ones", [P, 2], BF16, kind="ExternalInput")
    d["rms_bcast"] = nc.dram_tensor("rms_bcast", [2, P], BF16, kind="ExternalInput")
    d["ident"] = nc.dram_tensor("ident", [P, P], F32, kind="ExternalInput")
    for nm in ("gs1_pp", "gb1_pp", "gs2_pp", "gb2_pp", "ln1_pp", "ln2_pp", "ln3_pp"):
        d[nm] = nc.dram_tensor(nm, [P, NF], F32, kind="ExternalInput")
    for nm in ("rl1", "rl2", "rl3"):
        d[nm] = nc.dram_tensor(nm, [2, E], BF16, kind="ExternalInput")
    d["kqmap"] = nc.dram_tensor("kqmap", [E, S], BF16, kind="ExternalInput")
    d["kkmap"] = nc.dram_tensor("kkmap", [E, S], BF16, kind="ExternalInput")
    out_h = nc.dram_tensor("out", [S, E], F32, kind="ExternalOutput")

    with tile.TileContext(nc) as tc:
        _emit(nc, tc, d, out_h)
    _strip_self_waits(nc)
    _legalize_wait_counts(nc)
    return pr


_ENGINE_PROC = {
    "PE": "PE", "DVE": "DVE", "Activation": "Activation",
    "Pool": "Pool", "SP": "SP",
}


def _strip_self_waits(nc):
    """Remove same-engine sem waits on engine compute instructions.

    Engines execute their FIFO in order (DVE/ACT drain between ops; PE only
    reorders LDWEIGHTS pull-ahead, and PE never writes SBUF), so a wait on
    the instruction's own engine semaphore is redundant — and walrus only
    allows 2 sync waits per instruction."""
    import concourse.mybir as mb
    for f in nc.m.functions:
        for blk in f.blocks:
            for inst in blk.instructions:
                si = getattr(inst, "sync_info", None)
                if si is None or not si.on_wait:
                    continue
                tname = type(inst).__name__
                if tname in ("InstDMACopy", "InstDrain", "InstEventSemaphore",
                             "InstTriggerDma"):
                    continue
                eng = getattr(inst, "engine", None)
                eng_name = getattr(eng, "name", str(eng))
                pref = {"PE": "PE_", "DVE": "DVE_", "Activation": "Activation_",
                        "Pool": "Pool_", "SP": "SP_"}.get(eng_name)
                if not pref:
                    continue
                kept = [w for w in si.on_wait if not str(w.ant_name).startswith(pref)]
                if len(kept) != len(si.on_wait):
                    si.on_wait = kept


def _bc(row_ap, n_part):
    """Partition-broadcast read AP: replicate a single-partition row across
    n_part partitions (partition-step-0 source, for DMA)."""
    return bass.AP(tensor=row_ap.tensor, offset=row_ap.offset,
                   ap=[[0, n_part]] + [list(p) for p in row_ap.ap[1:]])


_MAX_WAITS = 1
_WAIT_BUDGET = {"InstActivation": 1, "InstDrain": 0}


def _legalize_wait_counts(nc):
    """walrus allows at most 2 sync waits per lowered instruction. Move any
    excess waits onto injected same-engine sequencer NOPs placed immediately
    before the offending instruction (program order on the engine's stream
    gates the instruction behind the NOP's waits)."""
    import bass_rust
    import concourse.mybir as mb
    uid = [0]
    for f in nc.m.functions:
        for blk in f.blocks:
            insts = list(blk.instructions)
            out = []
            changed = False
            for inst in insts:
                si = getattr(inst, "sync_info", None)
                waits = list(si.on_wait) if si and si.on_wait else []
                plain = [w for w in waits if w.sync_type == "semaphore"]
                other = [w for w in waits if w.sync_type != "semaphore"]
                cap = _WAIT_BUDGET.get(type(inst).__name__, _MAX_WAITS)
                if len(plain) + len(other) > cap and len(plain) > 0:
                    budget = max(0, cap - len(other))
                    keep, excess = plain[:budget], plain[budget:]
                    while excess:
                        chunk, excess = excess[:1], excess[1:]
                        nop = bass_rust.InstNoOp(name=f"wnop-{uid[0]}", ins=[], outs=[])
                        uid[0] += 1
                        nop.engine = inst.engine
                        nop.sync_info = mb.SyncInfo(on_wait=chunk, on_update=[])
                        out.append(nop)
                    si.on_wait = other + keep
                    changed = True
                out.append(inst)
            if changed:
                blk.instructions = out


def _emit(nc, tc, d, out_h):
    from contextlib import ExitStack
    ctx = ExitStack()
    with ctx:
        # Pools. Wait-limit discipline: every instruction may carry at most 2
        # sync waits after walrus lowering, so each tile has a single writer
        # engine and PSUM pools are split by evacuating engine (pg_d -> DVE,
        # pg_a -> ACT).
        p_const = ctx.enter_context(tc.tile_pool(name="const", bufs=1))
        p_act = ctx.enter_context(tc.tile_pool(name="act", bufs=1))
        p_w = ctx.enter_context(tc.tile_pool(name="w", bufs=12))
        p_map = ctx.enter_context(tc.tile_pool(name="map", bufs=8))
        p_sc = ctx.enter_context(tc.tile_pool(name="sc", bufs=8))
        p_sq = ctx.enter_context(tc.tile_pool(name="sq", bufs=2))
        p_sm = ctx.enter_context(tc.tile_pool(name="sm", bufs=8))
        # note: ld pool shares output tiles
        p_ld = ctx.enter_context(tc.tile_pool(name="ld", bufs=4))
        pg_d = ctx.enter_context(tc.tile_pool(name="pgd", bufs=2, space="PSUM"))
        pg_a = ctx.enter_context(tc.tile_pool(name="pga", bufs=2, space="PSUM"))
        psc = ctx.enter_context(tc.tile_pool(name="psc", bufs=3, space="PSUM"))
        pret = ctx.enter_context(tc.tile_pool(name="pret", bufs=1, space="PSUM"))

        # ---- constants ----
        cmask = p_const.tile([P, 4 * 512], BF16)
        nc.sync.dma_start(out=cmask, in_=d["cmask"][:, :])
        gn_ones = p_const.tile([P, 2], BF16)
        nc.sync.dma_start(out=gn_ones, in_=d["gn_ones"][:, :])
        gn_bcast = p_const.tile([2, P], BF16)
        nc.sync.dma_start(out=gn_bcast, in_=d["gn_bcast"][:, :])
        rms_ones = p_const.tile([P, 2], BF16)
        nc.sync.dma_start(out=rms_ones, in_=d["rms_ones"][:, :])
        rms_bcast = p_const.tile([2, P], BF16)
        nc.sync.dma_start(out=rms_bcast, in_=d["rms_bcast"][:, :])
        ident = p_const.tile([P, P], F32)
        nc.sync.dma_start(out=ident, in_=d["ident"][:, :])
        ppv = {}
        for nm in ("gs1_pp", "gb1_pp", "gs2_pp", "gb2_pp", "ln1_pp", "ln2_pp", "ln3_pp"):
            t = p_const.tile([P, NF], F32, name=nm)
            nc.sync.dma_start(out=t, in_=d[nm][:, :])
            ppv[nm] = t
        rlv = {}
        for nm in ("rl1", "rl2", "rl3"):
            t = p_const.tile([2, E], BF16, name=nm)
            nc.sync.dma_start(out=t, in_=d[nm][:, :])
            rlv[nm] = t
        eps_gn = p_const.tile([P, 1], F32)
        nc.vector.memset(eps_gn, 1e-5)
        eps_rms = p_const.tile([P, 1], F32)
        nc.vector.memset(eps_rms, 1e-6)
        zero_d = p_const.tile([P, 1], F32)
        nc.vector.memset(zero_d, 0.0)
        # DVE observers for HWDGE const queues (keeps later DVE ops <=2 waits)
        wuv = p_const.tile([P, 1], F32)
        for cn in list(ppv.values()) + [cmask]:
            nc.vector.tensor_copy(wuv, cn[:, 0:1])

        # Warmup matmuls so PE observes each PE-read constant's DMA queue sem
        # early (keeps later matmuls at <=2 waits).
        wu = pg_d.tile([P, P], F32, tag="pgd", name="wu")
        nc.tensor.matmul(wu[0:2, 0:P], gn_ones, cmask[:, 0:P], start=True, stop=True)
        nc.tensor.matmul(wu[0:P, 0:P], gn_bcast, gn_bcast, start=False, stop=True,
                         skip_group_check=True)
        nc.tensor.matmul(wu[0:2, 0:P], rms_ones, cmask[:, 0:P], start=False, stop=True,
                         skip_group_check=True)
        nc.tensor.matmul(wu[0:P, 0:P], rms_bcast, rms_bcast, start=False, stop=True,
                         skip_group_check=True)

        def load_w(nm, tag="w"):
            tiles = []
            for k in range(NF):
                wt = p_w.tile([P, E], BF16, tag=tag, name=f"{nm}_{k}")
                nc.sync.dma_start(out=wt, in_=d[nm][k * P:(k + 1) * P, :])
                tiles.append(wt)
            return tiles

        # ---- phase 0: load + transpose x, obs -> xT, obsT (evac: DVE only) ----
        def transpose_in(src_h, out_tag):
            outT = []
            for k in range(NF):
                t = p_act.tile([P, S], BF16, tag=f"{out_tag}{k}", name=f"{out_tag}{k}")
                outT.append(t)
            for sidx in range(NS):
                s_sb = p_ld.tile([P, E], F32, tag="ld", name=f"ld_{sidx}")
                nc.sync.dma_start(out=s_sb, in_=src_h[sidx * P:(sidx + 1) * P, :])
                ps = pg_d.tile([P, E], F32, tag="pgd", name=f"tp_{sidx}")
                for k in range(NF):
                    nc.tensor.matmul(ps[:, k * P:(k + 1) * P], s_sb[:, k * P:(k + 1) * P],
                                     ident, is_transpose=True,
                                     start=(k == 0), stop=(k == NF - 1))
                for k in range(NF):
                    nc.scalar.copy(outT[k][:, sidx * P:(sidx + 1) * P],
                                   ps[:, k * P:(k + 1) * P])
            return outT

        xT = transpose_in(d["x"], "xT")
        obsT = transpose_in(d["obs"], "obsT")

        # ---- helper: [E,E] gemm, out feature-major: outT = W^T @ srcT ----
        def gemm_fm(w_tiles, srcT, evac, out_tag=None, out_tiles=None, out_dt=BF16,
                    psum_pool=None):
            pool = psum_pool or pg_d
            outs = out_tiles
            if outs is None:
                outs = [p_act.tile([P, S], out_dt, tag=f"{out_tag}{m}", name=f"{out_tag}{m}")
                        for m in range(NF)]
            for m in range(NF):
                for nh in range(NH2):
                    ps = pool.tile([P, 512], F32, tag=pool.name, name=f"g_{m}_{nh}")
                    for k in range(NF):
                        nc.tensor.matmul(
                            ps, r(w_tiles[k][:, m * P:(m + 1) * P]),
                            r(srcT[k][:, nh * 512:(nh + 1) * 512]),
                            start=(k == 0), stop=(k == NF - 1))
                    evac(outs[m][:, nh * 512:(nh + 1) * 512], ps, m, nh)
            return outs

        def msr(qsrcT, kvsrcT, wq_t, wk_t, wv_t, wg_t, wo_t, gs_pp, gb_pp, out_tiles,
                tap="", dump_fm=None):
            # q^T / k^T: plain DVE evac, then in-place decay-map multiply
            # (maps precomputed on host, streamed from HBM).
            def mk_evac_map(map_h):
                def evac(dst, ps, m, nh):
                    mt = p_map.tile([P, 512], BF16, tag="map", name=f"map_{m}")
                    nc.sync.dma_start(
                        out=mt, in_=map_h[m * P:(m + 1) * P, nh * 512:(nh + 1) * 512])
                    nc.vector.tensor_mul(dst, ps, mt)
                return evac

            qT = gemm_fm(wq_t, qsrcT, mk_evac_map(d["kqmap"]), out_tag="qT")
            if tap == "qT":
                dump_fm(qT)
                return
            kT = gemm_fm(wk_t, kvsrcT, mk_evac_map(d["kkmap"]), out_tag="kT")
            if tap == "kT":
                dump_fm(kT)
                return

            # V seq-major: V[st] [128, 512(all heads)]
            V = []
            for st in range(NS):
                ps = pg_d.tile([P, 512], F32, tag="pgd", name=f"v_{st}")
                for k in range(NF):
                    nc.tensor.matmul(ps, r(kvsrcT[k][:, st * P:(st + 1) * P]), r(wv_t[k]),
                                     start=(k == 0), stop=(k == NF - 1))
                vt = p_act.tile([P, 512], BF16, tag=f"V{st}", name=f"V{st}")
                nc.scalar.copy(vt, ps)
                V.append(vt)
            if tap == "V":
                dump_fm(V, n_tiles=NS, width=E)
                return

            # scores + ret; two heads (one pair tile) share a ret psum bank:
            # even head -> rows 0:64, odd head -> rows 64:128 (col group).
            retT = [p_act.tile([P, S], BF16, tag=f"retT{pt}", name=f"retT{pt}") for pt in range(NF)]
            for pt in range(NF):
                for nt in range(NH2):
                    prt = pret.tile([P, 512], F32, tag="pret", name=f"pret_{pt}_{nt}")
                    groups = ([[0], [1], [2], [3]] if nt == 0
                              else [[0], [1], [2], [3], [4], [5], [6], [7]])
                    n_head_mm = sum(len(g) for g in groups)
                    for hh in range(2):      # head within pair
                        mm_i = 0
                        h = pt * 2 + hh
                        sl = hh * 64
                        for gi, grp in enumerate(groups):
                            ps4 = psc.tile([P, 512], F32, tag="psc", name=f"sc_{h}_{nt}_{gi}")
                            for j, mt in enumerate(grp):
                                # each j targets its own PSUM bank -> own group
                                nc.tensor.matmul(
                                    ps4[:, j * 512:(j + 1) * 512],
                                    r(kT[pt][sl:sl + 64, mt * P:(mt + 1) * P]),
                                    r(qT[pt][sl:sl + 64, nt * 512:(nt + 1) * 512]),
                                    start=True, stop=True)
                            sc_sb = p_sc.tile([P, 512], BF16, tag="scsb", name=f"scsb_{h}_{nt}_{gi}")
                            masked = (grp[-1] * P + P - 1) >= nt * 512
                            if masked:
                                for j, mt in enumerate(grp):
                                    off = mt * P - nt * 512   # 0/128/256/384
                                    s0 = j * 512
                                    if off > 0:
                                        nc.gpsimd.memset(sc_sb[:, s0:s0 + off], 0.0)
                                    # diagonal block: mask-multiply (cmask diag
                                    # of section i=off//128 is at abs col
                                    # i*512 + off)
                                    ci = (off // 128) * 512 + off
                                    nc.vector.tensor_mul(
                                        sc_sb[:, s0 + off:s0 + off + P],
                                        ps4[:, s0 + off:s0 + off + P],
                                        cmask[:, ci:ci + P])
                                    if off + P < 512:
                                        nc.scalar.copy(
                                            sc_sb[:, s0 + off + P:s0 + 512],
                                            ps4[:, s0 + off + P:s0 + 512])
                            else:
                                nc.scalar.copy(sc_sb, ps4)
                            for j, mt in enumerate(grp):
                                nc.tensor.matmul(
                                    prt[sl:sl + 64, :],
                                    r(V[mt][:, h * DH:(h + 1) * DH]),
                                    r(sc_sb[:, j * 512:(j + 1) * 512]),
                                    start=(mm_i == 0), stop=(mm_i == n_head_mm - 1),
                                    tile_position=(0, sl), skip_group_check=True)
                                mm_i += 1
                    nc.vector.tensor_copy(retT[pt][:, nt * 512:(nt + 1) * 512], prt)

            # GroupNorm (feature-major, stats over 64 partitions per head).
            for pt in range(NF):
                for nt in range(NH2):
                    rsl = retT[pt][:, nt * 512:(nt + 1) * 512]
                    sqt = p_sq.tile([P, 512], BF16, tag="gnsq", name=f"gnsq_{pt}_{nt}", bufs=2)
                    nc.gpsimd.tensor_mul(sqt, rsl, rsl)
                    pstat = pg_d.tile([P, 512], F32, tag="pgd", name=f"gst_{pt}_{nt}")
                    nc.tensor.matmul(pstat[0:2, :], r(gn_ones), r(rsl), start=True, stop=True)
                    pstat2 = pg_d.tile([P, 512], F32, tag="pgd", name=f"gst2_{pt}_{nt}")
                    nc.tensor.matmul(pstat2[0:2, :], r(gn_ones), r(sqt), start=True, stop=True)
                    mu = p_sm.tile([2, 512], BF16, tag="sm", name=f"mu_{pt}_{nt}")
                    mu2 = p_sm.tile([2, 512], F32, tag="sm", name=f"mu2_{pt}_{nt}")
                    var = p_sm.tile([2, 512], F32, tag="sm", name=f"var_{pt}_{nt}")
                    sd = p_sm.tile([2, 512], F32, tag="sm", name=f"sd_{pt}_{nt}")
                    rstd = p_sm.tile([2, 512], BF16, tag="sm", name=f"rstd_{pt}_{nt}")
                    nc.vector.tensor_copy(mu, pstat[0:2, :])
                    nc.vector.tensor_mul(mu2, mu, mu)
                    nc.vector.tensor_sub(var, pstat2[0:2, :], mu2)
                    nc.scalar.activation(sd, var, AF.Sqrt, bias=eps_gn[0:2, :])
                    with nc.allow_low_precision(reason="rstd feeds bf16 broadcast matmul"):
                        nc.vector.reciprocal(rstd, sd)
                    muBp = pg_d.tile([P, 512], F32, tag="pgd", name=f"muBp_{pt}_{nt}")
                    nc.tensor.matmul(muBp, r(gn_bcast), r(mu), start=True, stop=True)
                    rsBp = pg_d.tile([P, 512], F32, tag="pgd", name=f"rsBp_{pt}_{nt}")
                    nc.tensor.matmul(rsBp, r(gn_bcast), r(rstd), start=True, stop=True)
                    nc.vector.tensor_sub(rsl, rsl, muBp)
                    nc.vector.tensor_mul(rsl, rsl, rsBp)
                    nc.scalar.activation(rsl, rsl, AF.Identity,
                                         bias=gb_pp[:, pt:pt + 1], scale=gs_pp[:, pt:pt + 1])

            if tap == "ret":
                dump_fm(retT)
                return

            # gate: g^T = silu(Wg^T @ qsrcT); silu evac on ACT from pg_a
            def evac_g(dst, ps, m, nh):
                nc.scalar.activation(dst, ps, AF.Silu)

            gT = gemm_fm(wg_t, qsrcT, evac_g, out_tag="qT", psum_pool=pg_a)
            # gated = swish(g) * retGN, written into retT (PE reads retT for wo)
            for m in range(NF):
                nc.gpsimd.tensor_mul(retT[m], gT[m], retT[m])

            def evac_o(dst, ps, m, nh):
                nc.scalar.copy(dst, ps)

            gemm_fm(wo_t, retT, evac_o, out_tiles=out_tiles, psum_pool=pg_a)

        # feature-major RMSNorm: out = (a + b) * rsqrt(mean_f((a+b)^2) + eps) * ln
        def rms_fm(aT, bT, ln_pp, out_tag=None, out_tiles=None, out_dt=BF16):
            res = bT
            for k in range(NF):
                nc.vector.tensor_add(res[k], aT[k], bT[k])
            outs = out_tiles
            if outs is None:
                outs = [p_act.tile([P, S], out_dt, tag=f"{out_tag}{k}", name=f"{out_tag}{k}")
                        for k in range(NF)]
            for nh in range(NH2):
                pstat = pg_d.tile([P, 512], F32, tag="pgd", name=f"rst_{nh}")
                for k in range(NF):
                    sqt = p_sq.tile([P, 512], BF16, tag="sq", name=f"rsq_{nh}_{k}")
                    rs = res[k][:, nh * 512:(nh + 1) * 512]
                    nc.vector.tensor_mul(sqt, rs, rs)
                    nc.tensor.matmul(pstat[0:2, :], r(rms_ones), r(sqt),
                                     start=(k == 0), stop=(k == NF - 1))
                r0 = p_sm.tile([2, 512], F32, tag="sm", name=f"r0_{nh}")
                nc.vector.tensor_copy(r0[0:1, :], pstat[0:1, :])
                rA = p_sm.tile([2, 512], F32, tag="sm", name=f"rA_{nh}")
                nc.scalar.activation(rA[0:1, :], r0[0:1, :], AF.Sqrt,
                                     bias=eps_rms[0:1, :], scale=1.0 / E)
                rB = p_sm.tile([2, 512], BF16, tag="sm", name=f"rB_{nh}")
                nc.vector.memset(rB, 0.0)
                with nc.allow_low_precision(reason="rstd feeds bf16 broadcast matmul"):
                    nc.vector.reciprocal(rB[0:1, :], rA[0:1, :])
                rsB = pg_d.tile([P, 512], F32, tag="pgd", name=f"rmsB_{nh}")
                nc.tensor.matmul(rsB, r(rms_bcast), r(rB[0:2, :]), start=True, stop=True)
                for k in range(NF):
                    osl = outs[k][:, nh * 512:(nh + 1) * 512]
                    nc.vector.tensor_mul(osl, res[k][:, nh * 512:(nh + 1) * 512], rsB)
                    nc.gpsimd.tensor_scalar_mul(osl, osl, ln_pp[:, k:k + 1])
            return outs

        # ======== the block ========
        import os
        tap = os.environ.get("KTAP", "")

        def dump_fm(tiles, n_tiles=NF, width=S):
            # write feature-major tiles [128, width] into out rows sequentially
            for k in range(n_tiles):
                t32 = p_ld.tile([P, S], F32, tag="dump", name=f"dmp_{k}", bufs=2)
                nc.vector.tensor_copy(t32[:, :width], tiles[k][:, :width])
                rows = width // E
                for rr in range(rows):
                    nc.sync.dma_start(
                        out=out_h[(k * rows + rr) * P:(k * rows + rr + 1) * P, :],
                        in_=t32[:, rr * E:(rr + 1) * E])
        wq1 = load_w("wqc1"); wk1 = load_w("wkc1"); wv1 = load_w("wvc1")
        wg1 = load_w("wg1"); wo1 = load_w("wo1")
        msr1T = [p_act.tile([P, S], F32, tag=f"msrT{m}", name=f"msr1T{m}") for m in range(NF)]
        if tap == "xT":
            dump_fm(xT)
            return
        msr(xT, xT, wq1, wk1, wv1, wg1, wo1, ppv["gs1_pp"], ppv["gb1_pp"], msr1T,
            tap=tap, dump_fm=dump_fm)
        if tap:
            if tap == "msr1":
                dump_fm(msr1T)
            if tap in ("msr1", "qT", "kT", "V", "ret"):
                return
        x1T = rms_fm(xT, msr1T, ppv["ln1_pp"], out_tag="x1T")
        if tap == "x1":
            dump_fm(x1T)
            return

        wq2 = load_w("wqc2"); wk2 = load_w("wkc2"); wv2 = load_w("wvc2")
        wg2 = load_w("wg2"); wo2 = load_w("wo2")
        msr2T = [p_act.tile([P, S], F32, tag=f"msrT{m}", name=f"msr2T{m}") for m in range(NF)]
        msr(obsT, x1T, wq2, wk2, wv2, wg2, wo2, ppv["gs2_pp"], ppv["gb2_pp"], msr2T)
        x2T = rms_fm(obsT, msr2T, ppv["ln2_pp"], out_tag="xT")  # reuse xT slots

        def load_w_tags(nm, tags):
            tiles = []
            for k in range(NF):
                wt = p_act.tile([P, E], BF16, tag=tags[k], name=f"{nm}_{k}")
                nc.sync.dma_start(out=wt, in_=d[nm][k * P:(k + 1) * P, :])
                tiles.append(wt)
            return tiles

        wfg = load_w_tags("ffn_w_gate", [f"V{i}" for i in range(4)])
        wfl = load_w("ffn_w_lin")
        wfo = load_w("ffn_w_out")

        def evac_silu(dst, ps, m, nh):
            nc.scalar.activation(dst, ps, AF.Silu)

        def evac_cp_d(dst, ps, m, nh):
            nc.scalar.copy(dst, ps)

        fgT = gemm_fm(wfg, x2T, evac_silu, out_tag="qT", psum_pool=pg_a)
        flT = gemm_fm(wfl, x2T, evac_cp_d, out_tag="kT", psum_pool=pg_a)
        for m in range(NF):
            nc.gpsimd.tensor_mul(flT[m], fgT[m], flT[m])
        ffnT = gemm_fm(wfo, flT, evac_cp_d, out_tag="x1T", out_dt=F32, psum_pool=pg_a)
        x3T = rms_fm(x2T, ffnT, ppv["ln3_pp"], out_tag="msrT", out_dt=F32)

        # ---- output transpose: x3T [E,S] -> out [S,E] ----
        for sidx in range(NS):
            ps = pg_d.tile([P, E], F32, tag="pgd", name=f"ot_{sidx}")
            for k in range(NF):
                nc.tensor.matmul(ps[:, k * P:(k + 1) * P],
                                 x3T[k][:, sidx * P:(sidx + 1) * P], ident,
                                 is_transpose=True,
                                 start=(k == 0), stop=(k == NF - 1))
            ot = p_ld.tile([P, E], F32, tag="ot", name=f"ot_{sidx}", bufs=1)
            nc.vector.tensor_copy(ot, ps)
            nc.sync.dma_start(out=out_h[sidx * P:(sidx + 1) * P, :], in_=ot)


_prog_cache = None


def _get_program():
    global _prog_cache
    if _prog_cache is None:
        _prog_cache = _build_program()
    return _prog_cache


def kernel(**inputs):
    inputs = {k: np.asarray(v) for k, v in inputs.items()}
    consts = _build_consts(inputs)
    pr = _get_program()
    shared = dict(consts)
    x = np.ascontiguousarray(inputs["x"], dtype=np.float32)
    obs = np.ascontiguousarray(inputs["obs_rep"], dtype=np.float32)
    in_maps = []
    for b in range(N_CORES):
        m = dict(shared)
        m["x"] = np.ascontiguousarray(x[b])
        m["obs"] = np.ascontiguousarray(obs[b])
        in_maps.append(m)
    res = run_bass_kernel_spmd(pr.nc, in_maps, core_ids=list(range(N_CORES)))
    return np.stack([res.results[b]["out"] for b in range(N_CORES)], axis=0)



# revision 2
# speedup vs baseline: 1.0859x; 1.0859x over previous
"""Trainium2 Bass kernel for nn_DecodeBlock (RetNet-style decoder block), v2.

Sharding: data-parallel over batch (B=8) across 8 NeuronCores; no collectives.

Design (per core, vs the quadratic v1 baseline):
  - Chunked-recurrent retention (C=128): per chunk, intra-chunk scores
    [128,128] + cross-chunk contribution through a per-head [dk,dv] state
    accumulated in PSUM across chunks (global kappa^±n scaling keeps the
    recurrence a pure sum; exact, no approximation).
  - fp8-e4m3 DoubleRow matmuls (2 k-tiles per pass, 0.5 cyc/row) for the
    K_seq/gate/W_O/FFN gemms; q/k/V projections stay bf16 (precision).
  - Sequence-major normalization path: GroupNorm/RMSNorm stats as [128,8]
    narrow tiles (engine cost scales with free-size), ACT per-partition
    Rsqrt/scale application, residuals fused into gemm evacuations.
  - All transposes via the DMA XBAR (dma_start_transpose, bf16), not PE.
  - Output is produced sequence-major and DMA'd straight out.
"""

import numpy as np

import concourse.bass as bass
import concourse.mybir as mybir
import concourse.tile as tile
from concourse.bass_utils import run_bass_kernel_spmd

F32 = mybir.dt.float32
BF16 = mybir.dt.bfloat16
FP8 = mybir.dt.float8e4
AF = mybir.ActivationFunctionType
ALU = mybir.AluOpType
DRM = mybir.MatmulPerfMode.DoubleRow

E, H, B, S = 512, 8, 8, 1024
DH = E // H          # 64
P = 128
NF = E // P          # 4 feature tiles
NC = S // P          # 8 seq chunks

N_CORES = 8


def _kappas():
    k = 1.0 - np.exp(np.linspace(np.log(1.0 / 32.0), np.log(1.0 / 512.0), H))
    return k.astype(np.float64)


def _pair8(w):
    """[E, E] weight -> fp8 DR layout [128, 4*512]: col block j*512 = k-tile j
    (rows j*128..j*128+127)."""
    import ml_dtypes
    w = np.asarray(w, np.float32)
    return np.ascontiguousarray(
        w.reshape(NF, P, E).transpose(1, 0, 2).reshape(P, NF * E)
        .astype(ml_dtypes.float8_e4m3))


def _build_consts(inputs):
    import ml_dtypes
    bf16 = ml_dtypes.bfloat16
    kap = _kappas()
    n = np.arange(S, dtype=np.float64)
    kq = np.empty((E, S), np.float64)
    kk = np.empty((E, S), np.float64)
    for h in range(H):
        kq[h * DH:(h + 1) * DH, :] = (kap[h] ** n)[None, :]
        kk[h * DH:(h + 1) * DH, :] = (kap[h] ** (-n))[None, :]
    kks = np.empty((S, E), np.float64)   # seq-major kappa^-m, head-major cols
    for h in range(H):
        kks[:, h * DH:(h + 1) * DH] = (kap[h] ** (-n))[:, None]
    # causal keep n>=m, [128,128] repeated 4x along free
    cm = (np.arange(P)[None, :] >= np.arange(P)[:, None]).astype(np.float32)
    cmask4 = np.ascontiguousarray(np.tile(cm, (1, 4)).astype(bf16))

    ln1 = np.asarray(inputs["ln1_s"], np.float32)
    ln2 = np.asarray(inputs["ln2_s"], np.float32)

    def conc(w):
        return np.asarray(w, np.float32).transpose(1, 0, 2).reshape(E, E)

    wq1 = conc(inputs["wq1"]); wk1 = conc(inputs["wk1"]); wv1 = conc(inputs["wv1"])
    wq2 = conc(inputs["wq2"])
    wk2f = ln1[:, None] * conc(inputs["wk2"])   # fold ln1 into msr2 kv path
    wv2f = ln1[:, None] * conc(inputs["wv2"])
    fgf = ln2[:, None] * np.asarray(inputs["ffn_w_gate"], np.float32)
    flf = ln2[:, None] * np.asarray(inputs["ffn_w_lin"], np.float32)

    consts = {
        "kqm": np.ascontiguousarray(kq.astype(bf16)),
        "kkm": np.ascontiguousarray(kk.astype(bf16)),
        "kks": np.ascontiguousarray(kks.astype(bf16)),
        "cmask4": cmask4,
        "wqc1": np.ascontiguousarray(wq1.astype(bf16)),
        "wkc1": np.ascontiguousarray(wk1.astype(bf16)),
        "wvc1": np.ascontiguousarray(wv1.astype(bf16)),
        "wqc2": np.ascontiguousarray(wq2.astype(bf16)),
        "wkc2": np.ascontiguousarray(wk2f.astype(bf16)),
        "wvc2": np.ascontiguousarray(wv2f.astype(bf16)),
        "kp8_1": _pair8(wk1), "kp8_2": _pair8(wk2f),
        "wg8_1": _pair8(inputs["wg1"]), "wg8_2": _pair8(inputs["wg2"]),
        "wo8_1": _pair8(inputs["wo1"]), "wo8_2": _pair8(inputs["wo2"]),
        "fg8": _pair8(fgf), "fl8": _pair8(flf),
        "fo8": _pair8(inputs["ffn_w_out"]),
    }
    fl = _flags(inputs)
    if not fl[0]:
        consts["gsb1"] = np.ascontiguousarray(
            np.tile(np.asarray(inputs["gs1"], np.float32), (P, 1)))
        consts["gbb1"] = np.ascontiguousarray(
            np.tile(np.asarray(inputs["gb1"], np.float32), (P, 1)))
    if not fl[1]:
        consts["gsb2"] = np.ascontiguousarray(
            np.tile(np.asarray(inputs["gs2"], np.float32), (P, 1)))
        consts["gbb2"] = np.ascontiguousarray(
            np.tile(np.asarray(inputs["gb2"], np.float32), (P, 1)))
    if not fl[2]:
        consts["ln2C"] = np.ascontiguousarray(np.tile(ln2, (P, 1)))
    if not fl[3]:
        consts["ln3C"] = np.ascontiguousarray(
            np.tile(np.asarray(inputs["ln3_s"], np.float32), (P, 1)))
    return consts


def _flags(inputs):
    """(gn1 trivial, gn2 trivial, ln2 trivial, ln3 trivial)"""
    return (
        bool(np.allclose(inputs["gs1"], 1) and np.allclose(inputs["gb1"], 0)),
        bool(np.allclose(inputs["gs2"], 1) and np.allclose(inputs["gb2"], 0)),
        bool(np.allclose(inputs["ln2_s"], 1)),
        bool(np.allclose(inputs["ln3_s"], 1)),
    )


class _Prog:
    pass


def _strip_self_waits(nc):
    import concourse.mybir as mb
    for f in nc.m.functions:
        for blk in f.blocks:
            for inst in blk.instructions:
                si = getattr(inst, "sync_info", None)
                if si is None or not si.on_wait:
                    continue
                tname = type(inst).__name__
                if tname in ("InstDMACopy", "InstDrain", "InstEventSemaphore",
                             "InstTriggerDma", "InstDmaTransposeAnt"):
                    continue
                eng = getattr(inst, "engine", None)
                eng_name = getattr(eng, "name", str(eng))
                pref = {"PE": "PE_", "DVE": "DVE_", "Activation": "Activation_",
                        "Pool": "Pool_", "SP": "SP_"}.get(eng_name)
                if not pref:
                    continue
                kept = [w for w in si.on_wait if not str(w.ant_name).startswith(pref)]
                if len(kept) != len(si.on_wait):
                    si.on_wait = kept


_MAX_WAITS = 1
_WAIT_BUDGET = {"InstActivation": 1, "InstDrain": 0}


def _legalize_wait_counts(nc):
    import bass_rust
    import concourse.mybir as mb
    uid = [0]
    for f in nc.m.functions:
        for blk in f.blocks:
            insts = list(blk.instructions)
            out = []
            changed = False
            for inst in insts:
                si = getattr(inst, "sync_info", None)
                waits = list(si.on_wait) if si and si.on_wait else []
                plain = [w for w in waits if w.sync_type == "semaphore"]
                other = [w for w in waits if w.sync_type != "semaphore"]
                cap = _WAIT_BUDGET.get(type(inst).__name__, _MAX_WAITS)
                if len(plain) + len(other) > cap and len(plain) > 0:
                    budget = max(0, cap - len(other))
                    keep, excess = plain[:budget], plain[budget:]
                    while excess:
                        chunk, excess = excess[:1], excess[1:]
                        nop = bass_rust.InstNoOp(name=f"wnop-{uid[0]}", ins=[], outs=[])
                        uid[0] += 1
                        nop.engine = inst.engine
                        nop.sync_info = mb.SyncInfo(on_wait=chunk, on_update=[])
                        out.append(nop)
                    si.on_wait = other + keep
                    changed = True
                out.append(inst)
            if changed:
                blk.instructions = out


def _build_program(flags):
    nc = bass.Bass()
    pr = _Prog()
    pr.nc = nc
    d = {}
    d["x"] = nc.dram_tensor("x", [S, E], F32, kind="ExternalInput")
    d["obs"] = nc.dram_tensor("obs", [S, E], F32, kind="ExternalInput")
    for nm in ("wqc1", "wkc1", "wvc1", "wqc2", "wkc2", "wvc2"):
        d[nm] = nc.dram_tensor(nm, [E, E], BF16, kind="ExternalInput")
    for nm in ("kp8_1", "kp8_2", "wg8_1", "wg8_2", "wo8_1", "wo8_2",
               "fg8", "fl8", "fo8"):
        d[nm] = nc.dram_tensor(nm, [P, NF * E], FP8, kind="ExternalInput")
    d["kqm"] = nc.dram_tensor("kqm", [E, S], BF16, kind="ExternalInput")
    d["kkm"] = nc.dram_tensor("kkm", [E, S], BF16, kind="ExternalInput")
    d["kks"] = nc.dram_tensor("kks", [S, E], BF16, kind="ExternalInput")
    d["cmask4"] = nc.dram_tensor("cmask4", [P, 4 * P], BF16, kind="ExternalInput")
    gn1_triv, gn2_triv, ln2_triv, ln3_triv = flags
    if not gn1_triv:
        d["gsb1"] = nc.dram_tensor("gsb1", [P, E], F32, kind="ExternalInput")
        d["gbb1"] = nc.dram_tensor("gbb1", [P, E], F32, kind="ExternalInput")
    if not gn2_triv:
        d["gsb2"] = nc.dram_tensor("gsb2", [P, E], F32, kind="ExternalInput")
        d["gbb2"] = nc.dram_tensor("gbb2", [P, E], F32, kind="ExternalInput")
    if not ln2_triv:
        d["ln2C"] = nc.dram_tensor("ln2C", [P, E], F32, kind="ExternalInput")
    if not ln3_triv:
        d["ln3C"] = nc.dram_tensor("ln3C", [P, E], F32, kind="ExternalInput")
    out_h = nc.dram_tensor("out", [S, E], F32, kind="ExternalOutput")

    with tile.TileContext(nc) as tc:
        _emit(nc, tc, d, out_h, flags)
    _strip_self_waits(nc)
    _legalize_wait_counts(nc)
    return pr


def _ap3(t, off, d1, n1, d2, n2):
    """3D free AP over tile t: [partitions, (stride d1 x n1), (stride d2 x n2)]."""
    return bass.AP(tensor=t.tensor, offset=t.offset + off,
                   ap=[list(t.ap[0]), [d1, n1], [d2, n2]])


def _emit(nc, tc, d, out_h, flags):
    from contextlib import ExitStack
    gn1_triv, gn2_triv, ln2_triv, ln3_triv = flags
    import os
    tap = os.environ.get("KTAP", "")
    skips = set(os.environ.get("KSKIP", "").split(","))
    ctx = ExitStack()
    with ctx:
        p_c = ctx.enter_context(tc.tile_pool(name="const", bufs=1))
        p_w = ctx.enter_context(tc.tile_pool(name="w", bufs=2))
        p_w8 = ctx.enter_context(tc.tile_pool(name="w8", bufs=2))
        p_ld = ctx.enter_context(tc.tile_pool(name="ld", bufs=1))
        p_seq = ctx.enter_context(tc.tile_pool(name="seq", bufs=8))
        p_rot = ctx.enter_context(tc.tile_pool(name="rot", bufs=2))
        p_big = ctx.enter_context(tc.tile_pool(name="big", bufs=1))
        p_pair = ctx.enter_context(tc.tile_pool(name="pair", bufs=1))
        p_act = ctx.enter_context(tc.tile_pool(name="act", bufs=1))
        p_kv = ctx.enter_context(tc.tile_pool(name="kv", bufs=1))
        p_sc = ctx.enter_context(tc.tile_pool(name="scp", bufs=2))
        p_st = ctx.enter_context(tc.tile_pool(name="stp", bufs=1))
        p_sm = ctx.enter_context(tc.tile_pool(name="sm", bufs=4))
        p_res = ctx.enter_context(tc.tile_pool(name="res", bufs=2))
        pg = ctx.enter_context(tc.tile_pool(name="pg", bufs=2, space="PSUM"))
        psc = ctx.enter_context(tc.tile_pool(name="psc", bufs=2, space="PSUM"))
        pret = ctx.enter_context(tc.tile_pool(name="pret", bufs=2, space="PSUM"))
        pst = ctx.enter_context(tc.tile_pool(name="pst", bufs=1, space="PSUM"))
        pcro = ctx.enter_context(tc.tile_pool(name="pcro", bufs=1, space="PSUM"))

        # ---- consts ----
        cmask4 = p_c.tile([P, 4 * P], BF16)
        nc.sync.dma_start(out=cmask4, in_=d["cmask4"][:, :])
        eps_gn = p_c.tile([P, 1], F32)
        nc.vector.memset(eps_gn, 1e-5)
        eps_rms = p_c.tile([P, 1], F32)
        nc.vector.memset(eps_rms, 1e-6)
        gcons = {}
        for nm in ("gsb1", "gbb1", "gsb2", "gbb2", "ln2C", "ln3C"):
            if nm in d:
                t = p_c.tile([P, E], F32, name=nm)
                nc.sync.dma_start(out=t, in_=d[nm][:, :])
                gcons[nm] = t

        def load_wbf(nm):
            wt = p_w.tile([P, NF * E], BF16, tag=f"w{nm[1]}", name=nm)
            nc.sync.dma_start(
                out=wt, in_=d[nm][:, :].rearrange("(a p) e -> p a e", p=P))
            return wt

        def load_w8(nm, tag):
            wt = p_w8.tile([P, NF * E], FP8, tag=tag, name=nm)
            nc.sync.dma_start(out=wt, in_=d[nm][:, :])
            return wt

        def w8_rhs(wt, p):
            # rhs [128, 2, 512] for k-pair p
            return _ap3(wt, p * 2 * E, E, 2, 1, E)

        def w8_lhs(wt, p, m):
            # lhsT [128, 2, 128] for k-pair p, out col block m
            return _ap3(wt, p * 2 * E + m * P, E, 2, 1, P)

        def pair_lhs(pt_tile, p_idx, c):
            # activation pair tile [128, 2048]: k-tile i at cols i*1024
            return _ap3(pt_tile, c * P, S, 2, 1, P)

        def pair_rhs(pt_tile, nh):
            return _ap3(pt_tile, nh * 512, S, 2, 1, 512)

        def big_col(bt, k, c0, w):
            return bass.AP(tensor=bt.tensor, offset=bt.offset + k * S + c0,
                           ap=[list(bt.ap[0]), [1, w]])

        def bigT_ap(bt, c):
            # dma-transpose dest: [128, (S,4), (1,128)] at chunk col c
            return _ap3(bt, c * P, S, NF, 1, P)

        def grp(t, n=8, w=DH):
            return bass.AP(tensor=t.tensor, offset=t.offset,
                           ap=[list(t.ap[0]), [w, n], [1, w]])

        def bcast(t, n=8, w=DH):
            return bass.AP(tensor=t.tensor, offset=t.offset,
                           ap=[list(t.ap[0]), [1, n], [0, w]])

        # ---- input load + bf16 copy + dma-transpose ----
        def load_input(src_h, seq_tag, big_tag, pair_tag, q_alt):
            seqs = []
            big = p_big.tile([P, NF * S], BF16, tag=big_tag, name=big_tag)
            lds = []
            for hc in range(2):
                ld = p_ld.tile([P, 4 * E], F32, tag="ld", name=f"ld_{hc}")
                nc.sync.dma_start(
                    out=ld, in_=src_h[hc * 512:(hc + 1) * 512, :]
                    .rearrange("(a p) e -> p a e", p=P))
                lds.append(ld)
            for c in range(NC):
                sq = p_seq.tile([P, E], BF16, tag=seq_tag, name=f"{seq_tag}{c}")
                eng = nc.vector if c % 2 else nc.gpsimd
                eng.tensor_copy(
                    sq, lds[c // 4][:, (c % 4) * E:(c % 4 + 1) * E])
                seqs.append(sq)
                nc.scalar.dma_start_transpose(bigT_ap(big, c), sq)
            pairs = []
            for pi in range(2):
                pt = p_pair.tile([P, 2 * S], FP8, tag=f"{pair_tag}{pi}",
                                 name=f"{pair_tag}{pi}")
                for hf in range(2):
                    eng = nc.vector if (pi + hf) % 2 == 0 else nc.gpsimd
                    eng.tensor_copy(pt[:, hf * S:(hf + 1) * S],
                                    big[:, (pi * 2 + hf) * S:(pi * 2 + hf + 1) * S])
                pairs.append(pt)
            return seqs, big, pairs

        xb, xT, x8 = load_input(d["x"], "seqA", "bigA", "x8", 0)
        kqm_b = p_c.tile([P, NF * S], BF16, name="kqm_b")
        nc.sync.dma_start(
            out=kqm_b, in_=d["kqm"][:, :].rearrange("(a p) e -> p a e", p=P))
        kkm_b = p_c.tile([P, NF * S], BF16, name="kkm_b")
        nc.sync.dma_start(
            out=kkm_b, in_=d["kkm"][:, :].rearrange("(a p) e -> p a e", p=P))
        wq1t = load_wbf("wqc1"); wk1t = load_wbf("wkc1"); wv1t = load_wbf("wvc1")
        kp81 = load_w8("kp8_1", "kp8")
        ob, oT, o8 = load_input(d["obs"], "seqB", "bigB", "o8", 1)
        wg81 = load_w8("wg8_1", "wg8")
        wo81 = load_w8("wo8_1", "wo8")
        kksb = []
        for hc in range(2):
            kt = p_c.tile([P, 4 * E], BF16, name=f"kksb{hc}")
            nc.sync.dma_start(
                out=kt, in_=d["kks"][hc * 512:(hc + 1) * 512, :]
                .rearrange("(a p) e -> p a e", p=P))
            kksb.append(kt)
        wq2t = load_wbf("wqc2"); wk2t = load_wbf("wkc2"); wv2t = load_wbf("wvc2")
        kp82 = load_w8("kp8_2", "kp8"); wg82 = load_w8("wg8_2", "wg8")
        wo82 = load_w8("wo8_2", "wo8")
        fg8 = load_w8("fg8", "kp8"); fl8w = load_w8("fl8", "wg8")
        fo8 = load_w8("fo8", "wo8")

        def dump_seq(tiles, dt=F32):
            for c, t in enumerate(tiles[:NC]):
                o = p_ld.tile([P, E], F32, tag="dmp", name=f"dmp{c}")
                nc.vector.tensor_copy(o, t[:, :E])
                nc.sync.dma_start(out=out_h[c * P:(c + 1) * P, :], in_=o)

        def dump_fm(tiles, width=S):
            # feature-major tiles [128, width] -> out rows
            for k, t in enumerate(tiles[:NF]):
                o = p_ld.tile([P, S], F32, tag="dmpf", name=f"dmpf{k}", bufs=2)
                nc.vector.tensor_copy(o[:, :width], t[:, :width])
                rows = width // E
                for rr in range(rows):
                    nc.sync.dma_start(
                        out=out_h[(k * rows + rr) * P:(k * rows + rr + 1) * P, :],
                        in_=o[:, rr * E:(rr + 1) * E])

        # ================= MSR =================
        def msr(idx, qT_big, q8_pairs, kvT_big, kv8_pairs, resid_seq, wq_t, wk_t,
                wv_t, kp8, wg8, wo8, gn_triv, gsb, gbb, out_seq_tag, tap=""):
            sfx = str(idx)
            # --- q~ / k~ feature-major gemms (bf16) + decay-map evac ---
            qs = []
            ks = []
            for pt in range(NF):
                qs.append(p_act.tile([P, S], BF16, tag=f"qs{pt}", name=f"qs{sfx}_{pt}"))
                ks.append(p_act.tile([P, S], BF16, tag=f"ks{pt}", name=f"ks{sfx}_{pt}"))
            for pt in range(NF):
                for nh in range(2):
                    ps = pg.tile([P, 512], F32, tag="pg", name=f"q_{pt}_{nh}")
                    for k in range(NF):
                        nc.tensor.matmul(ps, wq_t[:, k * E + pt * P:
                                                   k * E + (pt + 1) * P],
                                         big_col(qT_big, k, nh * 512, 512),
                                         start=(k == 0), stop=(k == NF - 1))
                    nc.vector.tensor_mul(
                        qs[pt][:, nh * 512:(nh + 1) * 512], ps,
                        kqm_b[:, pt * S + nh * 512:pt * S + (nh + 1) * 512])
            for pt in range(NF):
                for nh in range(2):
                    ps = pg.tile([P, 512], F32, tag="pg", name=f"k_{pt}_{nh}")
                    for k in range(NF):
                        nc.tensor.matmul(ps, wk_t[:, k * E + pt * P:
                                                   k * E + (pt + 1) * P],
                                         big_col(kvT_big, k, nh * 512, 512),
                                         start=(k == 0), stop=(k == NF - 1))
                    nc.vector.tensor_mul(
                        ks[pt][:, nh * 512:(nh + 1) * 512], ps,
                        kkm_b[:, pt * S + nh * 512:pt * S + (nh + 1) * 512])
            if tap == "qs":
                dump_fm(qs)
                return None
            if tap == "ks":
                dump_fm(ks)
                return None

            # --- V (bf16) and K_seq (fp8 DR) seq-major gemms ---
            V = []
            Kq = []
            for c in range(NC):
                ps = pg.tile([P, 512], F32, tag="pg", name=f"v_{c}")
                for k in range(NF):
                    nc.tensor.matmul(ps, big_col(kvT_big, k, c * P, P),
                                     wv_t[:, k * E:(k + 1) * E],
                                     start=(k == 0), stop=(k == NF - 1))
                vt = p_kv.tile([P, 512], BF16, tag=f"V{c}", name=f"V{sfx}_{c}")
                nc.scalar.copy(vt, ps)
                V.append(vt)
            for c in range(NC):
                ps = pg.tile([P, 512], F32, tag="pg", name=f"kq_{c}")
                for pi in range(2):
                    nc.tensor.matmul(ps, pair_lhs(kv8_pairs[pi], pi, c),
                                     w8_rhs(kp8, pi), start=(pi == 0),
                                     stop=(pi == 1), perf_mode=DRM)
                kt = p_kv.tile([P, 512], BF16, tag=f"K{c}", name=f"K{sfx}_{c}")
                nc.vector.tensor_mul(
                    kt, ps, kksb[c // 4][:, (c % 4) * E:(c % 4 + 1) * E])
                Kq.append(kt)
            if tap == "V":
                dump_seq(V)
                return None
            if tap == "Kq":
                dump_seq(Kq)
                return None

            # --- gate gemm (fp8 DR, feature-major) + silu ---
            gfm = [p_act.tile([P, S], BF16, tag=f"gf{m}", name=f"gfm{sfx}_{m}")
                   for m in range(NF)]
            for m in ([] if "gate" in skips else range(NF)):
                for nh in range(2):
                    ps = pg.tile([P, 512], F32, tag="pg", name=f"g_{m}_{nh}")
                    for pi in range(2):
                        nc.tensor.matmul(ps, w8_lhs(wg8, pi, m),
                                         pair_rhs(q8_pairs[pi], nh),
                                         start=(pi == 0), stop=(pi == 1),
                                         perf_mode=DRM)
                    nc.scalar.activation(gfm[m][:, nh * 512:(nh + 1) * 512], ps,
                                         AF.Silu)

            # --- retention chunks (software-pipelined, batched GN stats) ---
            stc = []
            rn = []
            ret_sb = []
            sc_all = []
            sums_all = p_sm.tile([P, 64], F32, tag="sumsA", name=f"sumsA{sfx}",
                                 bufs=2)
            sqs_all = p_sm.tile([P, 64], F32, tag="sqsA", name=f"sqsA{sfx}",
                                bufs=2)

            def emit_scores(c):
                # bank parity = head parity so every matmul in a bank shares
                # the same tile_position row base (mixing row bases in one
                # bank breaks execution)
                sc_sb = []
                for par in range(2):
                    ps = psc.tile([P, 512], F32, tag="psc", name=f"sc_{c}_{par}")
                    sl = par * DH
                    for hh in range(4):
                        h = 2 * hh + par
                        ptt = h // 2
                        nc.tensor.matmul(
                            ps[:, hh * P:(hh + 1) * P],
                            ks[ptt][sl:sl + DH, c * P:(c + 1) * P],
                            qs[ptt][sl:sl + DH, c * P:(c + 1) * P],
                            start=(hh == 0), stop=(hh == 3),
                            skip_group_check=True)
                    sb = p_sc.tile([P, 512], BF16, tag=f"scsb{par}",
                                   name=f"scsb_{c}_{par}")
                    nc.vector.tensor_mul(sb, ps, cmask4)
                    sc_sb.append(sb)
                sc_all.append(sc_sb)

            def emit_stage(c):
                # state update first so chunk c+1's cross input is ready early
                if c < NC - 1 and "state" not in skips:
                    pstt = pst.tile([P, 256], F32, tag="pst", name=f"st_{c}",
                                    bufs=1)
                    for h in range(H):
                        ptt, sl = h // 2, (h % 2) * DH
                        nc.tensor.matmul(
                            pstt[sl:sl + DH, ptt * DH:(ptt + 1) * DH],
                            Kq[c][:, h * DH:(h + 1) * DH],
                            V[c][:, h * DH:(h + 1) * DH],
                            start=(h < 2), stop=(h >= H - 2),
                            skip_group_check=True, tile_position=(0, sl))
                    st = p_sm.tile([P, 256], BF16, tag="stc", name=f"stc_{c}",
                                   bufs=2)
                    if c == 0:
                        nc.vector.tensor_copy(st, pstt)
                    else:
                        nc.vector.tensor_add(st, pstt, stc[c - 1])
                    stc.append(st)
                prt = pret.tile([P, 512], F32, tag="pret", name=f"ret_{c}")
                cross = (c > 0) and ("state" not in skips) and \
                    ("intra" not in skips)
                for h in ([] if "intra" in skips else range(H)):
                    nc.tensor.matmul(
                        prt[:, h * DH:(h + 1) * DH],
                        sc_all[c][h % 2][:, (h // 2) * P:(h // 2 + 1) * P],
                        V[c][:, h * DH:(h + 1) * DH],
                        start=(h == 0), stop=(h == H - 1 and not cross),
                        skip_group_check=True)
                if cross:
                    for h in range(0, H, 2):       # even heads: row base 0
                        ptt = h // 2
                        nc.tensor.matmul(
                            prt[:, h * DH:(h + 1) * DH],
                            qs[ptt][0:DH, c * P:(c + 1) * P],
                            stc[c - 1][0:DH, ptt * DH:(ptt + 1) * DH],
                            start=False, stop=(h == H - 2),
                            skip_group_check=True)
                    pco = pcro.tile([P, 256], F32, tag="pcro", name=f"cro_{c}")
                    for h in range(1, H, 2):       # odd heads: row base 64
                        ptt = h // 2
                        nc.tensor.matmul(
                            pco[:, ptt * DH:(ptt + 1) * DH],
                            qs[ptt][DH:2 * DH, c * P:(c + 1) * P],
                            stc[c - 1][DH:2 * DH, ptt * DH:(ptt + 1) * DH],
                            start=(h == 1), stop=(h == H - 1),
                            skip_group_check=True)
                rb = p_seq.tile([P, 512], BF16, tag="retsb", name=f"retsb_{c}")
                if "intra" in skips:
                    nc.vector.memset(rb, 0.0)
                else:
                    nc.scalar.copy(rb, prt)
                    if cross:
                        def _odd(t, w):
                            return bass.AP(tensor=t.tensor,
                                           offset=t.offset + (DH if w else 0),
                                           ap=[list(t.ap[0]),
                                               [2 * DH if w else DH, 4],
                                               [1, DH]])
                        nc.vector.tensor_add(_odd(rb, True), _odd(rb, True),
                                             _odd(pco, False))
                ret_sb.append(rb)
                if "gn" not in skips:
                    sq = p_rot.tile([P, 512], BF16, tag="gnsq",
                                    name=f"gnsq_{c}")
                    nc.scalar.activation(sq, rb, AF.Square)
                    nc.vector.tensor_reduce(sums_all[:, c * H:(c + 1) * H],
                                            grp(rb),
                                            axis=mybir.AxisListType.X,
                                            op=ALU.add)
                    nc.vector.tensor_reduce(sqs_all[:, c * H:(c + 1) * H],
                                            grp(sq),
                                            axis=mybir.AxisListType.X,
                                            op=ALU.add)

            import os as _os
            if _os.environ.get("KNOPIPE"):
                for c in range(NC):
                    emit_scores(c)
                    emit_stage(c)
            else:
                emit_scores(0)
                for c in range(1, NC):
                    emit_scores(c)
                    emit_stage(c - 1)
                emit_stage(NC - 1)
            if tap == "ret":
                dump_seq(ret_sb)
                return None

            # batched narrow stats for all (chunk, head)
            mu = p_sm.tile([P, 64], F32, tag="mu", name=f"mu{sfx}", bufs=1)
            nc.vector.tensor_scalar_mul(mu, sums_all, 1.0 / DH)
            m2 = p_sm.tile([P, 64], F32, tag="m2", name=f"m2{sfx}", bufs=1)
            nc.gpsimd.tensor_mul(m2, mu, mu)
            msq = p_sm.tile([P, 64], F32, tag="msq", name=f"msq{sfx}", bufs=1)
            nc.gpsimd.tensor_scalar_mul(msq, sqs_all, 1.0 / DH)
            var = p_sm.tile([P, 64], F32, tag="var", name=f"var{sfx}", bufs=1)
            nc.vector.tensor_sub(var, msq, m2)
            sd = p_sm.tile([P, 64], F32, tag="sd", name=f"sd{sfx}", bufs=1)
            nc.scalar.activation(sd, var, AF.Sqrt, bias=eps_gn)
            rs = p_sm.tile([P, 64], F32, tag="rs", name=f"rs{sfx}", bufs=1)
            nc.vector.reciprocal(rs, sd)
            c2 = p_sm.tile([P, 64], F32, tag="c2", name=f"c2{sfx}", bufs=1)
            nc.gpsimd.tensor_mul(c2, mu, rs)

            def bc8(t, c):
                return bass.AP(tensor=t.tensor, offset=t.offset + c * H,
                               ap=[list(t.ap[0]), [1, H], [0, DH]])

            for c in range(NC):
                tmp = p_rot.tile([P, 512], BF16, tag="gntmp", name=f"gntmp_{c}")
                nc.gpsimd.tensor_mul(grp(tmp), grp(ret_sb[c]), bc8(rs, c))
                rt = p_rot.tile([P, 512], BF16, tag="rn", name=f"rn_{c}", bufs=2)
                nc.gpsimd.tensor_sub(grp(rt), grp(tmp), bc8(c2, c))
                if not gn_triv:
                    nc.vector.tensor_mul(rt, rt, gsb)
                    nc.vector.tensor_add(rt, rt, gbb)
                rn.append(rt)
                if tap != "rnnt":
                    nc.scalar.dma_start_transpose(bigT_ap(rnT, c), rt)
            if tap in ("rn", "rnnt"):
                dump_seq(rn)
                return None

            # --- gating (feature-major) -> fp8 pair tiles ---
            g8 = [p_pair.tile([P, 2 * S], FP8, tag=f"gated{pi}",
                              name=f"gated{sfx}_{pi}") for pi in range(2)]
            for m in range(NF):
                nc.gpsimd.tensor_mul(
                    g8[m // 2][:, (m % 2) * S:(m % 2 + 1) * S],
                    gfm[m], rnT[:, m * S:(m + 1) * S])

            # --- W_O gemm (fp8 DR, seq-major out) + residual + RMSNorm ---
            outs = []
            for c in range(NC):
                ps = pg.tile([P, 512], F32, tag="pg", name=f"wo_{c}")
                for pi in range(2):
                    nc.tensor.matmul(ps, pair_lhs(g8[pi], pi, c),
                                     w8_rhs(wo8, pi), start=(pi == 0),
                                     stop=(pi == 1), perf_mode=DRM)
                res = p_res.tile([P, E], F32, tag="res", name=f"res{sfx}_{c}")
                nc.vector.tensor_add(res, ps, resid_seq[c])
                ssq = p_sm.tile([P, 1], F32, tag="ssq", name=f"ssq_{c}", bufs=2)
                ts = p_rot.tile([P, E], BF16, tag="gnsq", name=f"ttr_{c}")
                nc.scalar.activation(ts, res, AF.Square)
                nc.vector.tensor_reduce(ssq, ts, axis=mybir.AxisListType.X,
                                        op=ALU.add)
                sdr = p_sm.tile([P, 1], F32, tag="sdr", name=f"sdr_{c}", bufs=2)
                nc.scalar.activation(sdr, ssq, AF.Sqrt, bias=eps_rms,
                                     scale=1.0 / E)
                rsr = p_sm.tile([P, 1], F32, tag="rsr", name=f"rsr_{c}", bufs=2)
                nc.vector.reciprocal(rsr, sdr)
                o = p_seq.tile([P, E], BF16, tag=out_seq_tag,
                               name=f"{out_seq_tag}{c}")
                nc.scalar.activation(o, res, AF.Identity, scale=rsr)
                outs.append(o)
            return outs

        # rnT big tile shared by both msrs (rotates)
        rnT = p_big.tile([P, NF * S], BF16, tag="bigR", name="rnT1")
        r = msr(1, xT, x8, xT, x8, xb, wq1t, wk1t,
                wv1t, kp81, wg81, wo81,
                gn1_triv, gcons.get("gsb1"), gcons.get("gbb1"), "seqA",
                tap=tap if tap in ("qs", "ks", "V", "Kq", "rn", "ret", "rnnt") else "")
        if tap in ("qs", "ks", "V", "Kq", "rn", "ret", "rnnt"):
            return
        x1 = r
        if tap == "x1":
            dump_seq(x1)
            return

        # x1 -> feature-major (dma transpose) + fp8 pairs
        x1T = p_big.tile([P, NF * S], BF16, tag="bigA", name="x1T")
        for c in range(NC):
            nc.scalar.dma_start_transpose(bigT_ap(x1T, c), x1[c])
        x18 = []
        for pi in range(2):
            pt = p_pair.tile([P, 2 * S], FP8, tag=f"x8{pi}", name=f"x18_{pi}")
            for hf in range(2):
                eng = nc.vector if (pi + hf) % 2 == 0 else nc.gpsimd
                eng.tensor_copy(pt[:, hf * S:(hf + 1) * S],
                                x1T[:, (pi * 2 + hf) * S:(pi * 2 + hf + 1) * S])
            x18.append(pt)

        rnT = p_big.tile([P, NF * S], BF16, tag="bigR", name="rnT2")
        r = msr(2, oT, o8, x1T, x18, ob, wq2t, wk2t,
                wv2t, kp82, wg82, wo82,
                gn2_triv, gcons.get("gsb2"), gcons.get("gbb2"), "seqC")
        x2 = r
        if not ln2_triv:
            for c in range(NC):
                nc.gpsimd.tensor_mul(x2[c], x2[c], gcons["ln2C"])
        if tap == "x2":
            dump_seq(x2)
            return

        # x2 -> feature-major + fp8 pairs (ffn inputs)
        x2T = p_big.tile([P, NF * S], BF16, tag="bigR", name="x2T")
        for c in range(NC):
            nc.scalar.dma_start_transpose(bigT_ap(x2T, c), x2[c])
        x28 = []
        for pi in range(2):
            pt = p_pair.tile([P, 2 * S], FP8, tag=f"o8{pi}", name=f"x28_{pi}")
            for hf in range(2):
                eng = nc.vector if (pi + hf) % 2 == 0 else nc.gpsimd
                eng.tensor_copy(pt[:, hf * S:(hf + 1) * S],
                                x2T[:, (pi * 2 + hf) * S:(pi * 2 + hf + 1) * S])
            x28.append(pt)

        # ---- FFN (all fp8 DR) ----
        ffg = [p_act.tile([P, S], BF16, tag=f"qs{m}", name=f"ffg_{m}")
               for m in range(NF)]
        ffl = [p_act.tile([P, S], BF16, tag=f"ks{m}", name=f"ffl_{m}")
               for m in range(NF)]
        for m in range(NF):
            for nh in range(2):
                ps = pg.tile([P, 512], F32, tag="pg", name=f"fg_{m}_{nh}")
                for pi in range(2):
                    nc.tensor.matmul(ps, w8_lhs(fg8, pi, m),
                                     pair_rhs(x28[pi], nh), start=(pi == 0),
                                     stop=(pi == 1), perf_mode=DRM)
                nc.scalar.activation(ffg[m][:, nh * 512:(nh + 1) * 512], ps,
                                     AF.Silu)
        for m in range(NF):
            for nh in range(2):
                ps = pg.tile([P, 512], F32, tag="pg", name=f"fl_{m}_{nh}")
                for pi in range(2):
                    nc.tensor.matmul(ps, w8_lhs(fl8w, pi, m),
                                     pair_rhs(x28[pi], nh), start=(pi == 0),
                                     stop=(pi == 1), perf_mode=DRM)
                nc.vector.tensor_copy(ffl[m][:, nh * 512:(nh + 1) * 512], ps)
        fl8t = [p_pair.tile([P, 2 * S], FP8, tag=f"gated{pi}", name=f"flT8_{pi}")
                for pi in range(2)]
        for m in range(NF):
            nc.gpsimd.tensor_mul(fl8t[m // 2][:, (m % 2) * S:(m % 2 + 1) * S],
                                 ffg[m], ffl[m])
        for c in range(NC):
            ps = pg.tile([P, 512], F32, tag="pg", name=f"fo_{c}")
            for pi in range(2):
                nc.tensor.matmul(ps, pair_lhs(fl8t[pi], pi, c),
                                 w8_rhs(fo8, pi), start=(pi == 0),
                                 stop=(pi == 1), perf_mode=DRM)
            res = p_res.tile([P, E], F32, tag="res", name=f"res3_{c}")
            nc.vector.tensor_add(res, ps, x2[c])
            ssq = p_sm.tile([P, 1], F32, tag="ssq", name=f"ssq3_{c}", bufs=2)
            ts = p_rot.tile([P, E], BF16, tag="gnsq", name=f"ttr3_{c}")
            nc.scalar.activation(ts, res, AF.Square)
            nc.vector.tensor_reduce(ssq, ts, axis=mybir.AxisListType.X,
                                    op=ALU.add)
            sdr = p_sm.tile([P, 1], F32, tag="sdr", name=f"sdr3_{c}", bufs=2)
            nc.scalar.activation(sdr, ssq, AF.Sqrt, bias=eps_rms, scale=1.0 / E)
            rsr = p_sm.tile([P, 1], F32, tag="rsr", name=f"rsr3_{c}", bufs=2)
            nc.vector.reciprocal(rsr, sdr)
            if c % 4 == 0:
                obig = p_ld.tile([P, 4 * E], F32, tag="oo", name=f"oo_{c // 4}",
                                 bufs=1)
            o = obig[:, (c % 4) * E:(c % 4 + 1) * E]
            nc.scalar.activation(o, res, AF.Identity, scale=rsr)
            if not ln3_triv:
                nc.gpsimd.tensor_mul(o, o, gcons["ln3C"])
            if c % 4 == 3:
                nc.sync.dma_start(
                    out=out_h[(c - 3) * P:(c + 1) * P, :]
                    .rearrange("(a p) e -> p a e", p=P), in_=obig)


_prog_cache = {}


def _get_program(flags=(True, True, True, True)):
    if flags not in _prog_cache:
        _prog_cache[flags] = _build_program(flags)
    return _prog_cache[flags]


def kernel(**inputs):
    inputs = {k: np.asarray(v) for k, v in inputs.items()}
    flags = _flags(inputs)
    consts = _build_consts(inputs)
    pr = _get_program(flags)
    x = np.ascontiguousarray(inputs["x"], dtype=np.float32)
    obs = np.ascontiguousarray(inputs["obs_rep"], dtype=np.float32)
    in_maps = []
    for b in range(N_CORES):
        m = dict(consts)
        m["x"] = np.ascontiguousarray(x[b])
        m["obs"] = np.ascontiguousarray(obs[b])
        in_maps.append(m)
    res = run_bass_kernel_spmd(pr.nc, in_maps, core_ids=list(range(N_CORES)))
    return np.stack([res.results[b]["out"] for b in range(N_CORES)], axis=0)


# revision 3
# speedup vs baseline: 1.1247x; 1.0357x over previous
"""Trainium2 Bass kernel for nn_DecodeBlock (RetNet-style decoder block), v2.

Sharding: data-parallel over batch (B=8) across 8 NeuronCores; no collectives.

Design (per core, vs the quadratic v1 baseline):
  - Chunked-recurrent retention (C=128): per chunk, intra-chunk scores
    [128,128] + cross-chunk contribution through a per-head [dk,dv] state
    accumulated in PSUM across chunks (global kappa^±n scaling keeps the
    recurrence a pure sum; exact, no approximation).
  - fp8-e4m3 DoubleRow matmuls (2 k-tiles per pass, 0.5 cyc/row) for the
    K_seq/gate/W_O/FFN gemms; q/k/V projections stay bf16 (precision).
  - Sequence-major normalization path: GroupNorm/RMSNorm stats as [128,8]
    narrow tiles (engine cost scales with free-size), ACT per-partition
    Rsqrt/scale application, residuals fused into gemm evacuations.
  - All transposes via the DMA XBAR (dma_start_transpose, bf16), not PE.
  - Output is produced sequence-major and DMA'd straight out.
"""

import numpy as np

import concourse.bass as bass
import concourse.mybir as mybir
import concourse.tile as tile
from concourse.bass_utils import run_bass_kernel_spmd

F32 = mybir.dt.float32
BF16 = mybir.dt.bfloat16
FP8 = mybir.dt.float8e4
AF = mybir.ActivationFunctionType
ALU = mybir.AluOpType
DRM = mybir.MatmulPerfMode.DoubleRow

E, H, B, S = 512, 8, 8, 1024
DH = E // H          # 64
P = 128
NF = E // P          # 4 feature tiles
NC = S // P          # 8 seq chunks

N_CORES = 8


def _kappas():
    k = 1.0 - np.exp(np.linspace(np.log(1.0 / 32.0), np.log(1.0 / 512.0), H))
    return k.astype(np.float64)


def _pair8(w):
    """[E, E] weight -> fp8 DR layout [128, 4*512]: col block j*512 = k-tile j
    (rows j*128..j*128+127)."""
    import ml_dtypes
    w = np.asarray(w, np.float32)
    return np.ascontiguousarray(
        w.reshape(NF, P, E).transpose(1, 0, 2).reshape(P, NF * E)
        .astype(ml_dtypes.float8_e4m3))


def _build_consts(inputs):
    import ml_dtypes
    bf16 = ml_dtypes.bfloat16
    kap = _kappas()
    n = np.arange(S, dtype=np.float64)
    kq = np.empty((E, S), np.float64)
    kk = np.empty((E, S), np.float64)
    for h in range(H):
        kq[h * DH:(h + 1) * DH, :] = (kap[h] ** n)[None, :]
        kk[h * DH:(h + 1) * DH, :] = (kap[h] ** (-n))[None, :]
    kks = np.empty((S, E), np.float64)   # seq-major kappa^-m, head-major cols
    for h in range(H):
        kks[:, h * DH:(h + 1) * DH] = (kap[h] ** (-n))[:, None]
    # causal keep n>=m, [128,128] repeated 4x along free
    cm = (np.arange(P)[None, :] >= np.arange(P)[:, None]).astype(np.float32)
    cmask4 = np.ascontiguousarray(np.tile(cm, (1, 4)).astype(bf16))

    ln1 = np.asarray(inputs["ln1_s"], np.float32)
    ln2 = np.asarray(inputs["ln2_s"], np.float32)

    def conc(w):
        return np.asarray(w, np.float32).transpose(1, 0, 2).reshape(E, E)

    wq1 = conc(inputs["wq1"]); wk1 = conc(inputs["wk1"]); wv1 = conc(inputs["wv1"])
    wq2 = conc(inputs["wq2"])
    wk2f = ln1[:, None] * conc(inputs["wk2"])   # fold ln1 into msr2 kv path
    wv2f = ln1[:, None] * conc(inputs["wv2"])
    fgf = ln2[:, None] * np.asarray(inputs["ffn_w_gate"], np.float32)
    flf = ln2[:, None] * np.asarray(inputs["ffn_w_lin"], np.float32)

    consts = {
        "kqm": np.ascontiguousarray(kq.astype(bf16)),
        "kkm": np.ascontiguousarray(kk.astype(bf16)),
        "kks": np.ascontiguousarray(kks.astype(bf16)),
        "cmask4": cmask4,
        "wqc1": np.ascontiguousarray(wq1.astype(bf16)),
        "wkc1": np.ascontiguousarray(wk1.astype(bf16)),
        "wvc1": np.ascontiguousarray(wv1.astype(bf16)),
        "wqc2": np.ascontiguousarray(wq2.astype(bf16)),
        "wkc2": np.ascontiguousarray(wk2f.astype(bf16)),
        "wvc2": np.ascontiguousarray(wv2f.astype(bf16)),
        "kp8_1": _pair8(wk1), "kp8_2": _pair8(wk2f),
        "wg8_1": _pair8(inputs["wg1"]), "wg8_2": _pair8(inputs["wg2"]),
        "wo8_1": _pair8(inputs["wo1"]), "wo8_2": _pair8(inputs["wo2"]),
        "fg8": _pair8(fgf), "fl8": _pair8(flf),
        "fo8": _pair8(inputs["ffn_w_out"]),
    }
    fl = _flags(inputs)
    if not fl[0]:
        consts["gsb1"] = np.ascontiguousarray(
            np.tile(np.asarray(inputs["gs1"], np.float32), (P, 1)))
        consts["gbb1"] = np.ascontiguousarray(
            np.tile(np.asarray(inputs["gb1"], np.float32), (P, 1)))
    if not fl[1]:
        consts["gsb2"] = np.ascontiguousarray(
            np.tile(np.asarray(inputs["gs2"], np.float32), (P, 1)))
        consts["gbb2"] = np.ascontiguousarray(
            np.tile(np.asarray(inputs["gb2"], np.float32), (P, 1)))
    if not fl[2]:
        consts["ln2C"] = np.ascontiguousarray(np.tile(ln2, (P, 1)))
    if not fl[3]:
        consts["ln3C"] = np.ascontiguousarray(
            np.tile(np.asarray(inputs["ln3_s"], np.float32), (P, 1)))
    return consts


def _flags(inputs):
    """(gn1 trivial, gn2 trivial, ln2 trivial, ln3 trivial)"""
    return (
        bool(np.allclose(inputs["gs1"], 1) and np.allclose(inputs["gb1"], 0)),
        bool(np.allclose(inputs["gs2"], 1) and np.allclose(inputs["gb2"], 0)),
        bool(np.allclose(inputs["ln2_s"], 1)),
        bool(np.allclose(inputs["ln3_s"], 1)),
    )


class _Prog:
    pass


def _strip_self_waits(nc):
    import concourse.mybir as mb
    for f in nc.m.functions:
        for blk in f.blocks:
            for inst in blk.instructions:
                si = getattr(inst, "sync_info", None)
                if si is None or not si.on_wait:
                    continue
                tname = type(inst).__name__
                if tname in ("InstDMACopy", "InstDrain", "InstEventSemaphore",
                             "InstTriggerDma", "InstDmaTransposeAnt"):
                    continue
                eng = getattr(inst, "engine", None)
                eng_name = getattr(eng, "name", str(eng))
                pref = {"PE": "PE_", "DVE": "DVE_", "Activation": "Activation_",
                        "Pool": "Pool_", "SP": "SP_"}.get(eng_name)
                if not pref:
                    continue
                kept = [w for w in si.on_wait if not str(w.ant_name).startswith(pref)]
                if len(kept) != len(si.on_wait):
                    si.on_wait = kept


_MAX_WAITS = 1
_WAIT_BUDGET = {"InstActivation": 1, "InstDrain": 0}


def _legalize_wait_counts(nc):
    import bass_rust
    import concourse.mybir as mb
    uid = [0]
    for f in nc.m.functions:
        for blk in f.blocks:
            insts = list(blk.instructions)
            out = []
            changed = False
            for inst in insts:
                si = getattr(inst, "sync_info", None)
                waits = list(si.on_wait) if si and si.on_wait else []
                plain = [w for w in waits if w.sync_type == "semaphore"]
                other = [w for w in waits if w.sync_type != "semaphore"]
                cap = _WAIT_BUDGET.get(type(inst).__name__, _MAX_WAITS)
                if len(plain) + len(other) > cap and len(plain) > 0:
                    budget = max(0, cap - len(other))
                    keep, excess = plain[:budget], plain[budget:]
                    while excess:
                        chunk, excess = excess[:1], excess[1:]
                        nop = bass_rust.InstNoOp(name=f"wnop-{uid[0]}", ins=[], outs=[])
                        uid[0] += 1
                        nop.engine = inst.engine
                        nop.sync_info = mb.SyncInfo(on_wait=chunk, on_update=[])
                        out.append(nop)
                    si.on_wait = other + keep
                    changed = True
                out.append(inst)
            if changed:
                blk.instructions = out


def _build_program(flags):
    nc = bass.Bass()
    pr = _Prog()
    pr.nc = nc
    d = {}
    d["x"] = nc.dram_tensor("x", [S, E], F32, kind="ExternalInput")
    d["obs"] = nc.dram_tensor("obs", [S, E], F32, kind="ExternalInput")
    for nm in ("wqc1", "wkc1", "wvc1", "wqc2", "wkc2", "wvc2"):
        d[nm] = nc.dram_tensor(nm, [E, E], BF16, kind="ExternalInput")
    for nm in ("kp8_1", "kp8_2", "wg8_1", "wg8_2", "wo8_1", "wo8_2",
               "fg8", "fl8", "fo8"):
        d[nm] = nc.dram_tensor(nm, [P, NF * E], FP8, kind="ExternalInput")
    d["kqm"] = nc.dram_tensor("kqm", [E, S], BF16, kind="ExternalInput")
    d["kkm"] = nc.dram_tensor("kkm", [E, S], BF16, kind="ExternalInput")
    d["kks"] = nc.dram_tensor("kks", [S, E], BF16, kind="ExternalInput")
    d["cmask4"] = nc.dram_tensor("cmask4", [P, 4 * P], BF16, kind="ExternalInput")
    gn1_triv, gn2_triv, ln2_triv, ln3_triv = flags
    if not gn1_triv:
        d["gsb1"] = nc.dram_tensor("gsb1", [P, E], F32, kind="ExternalInput")
        d["gbb1"] = nc.dram_tensor("gbb1", [P, E], F32, kind="ExternalInput")
    if not gn2_triv:
        d["gsb2"] = nc.dram_tensor("gsb2", [P, E], F32, kind="ExternalInput")
        d["gbb2"] = nc.dram_tensor("gbb2", [P, E], F32, kind="ExternalInput")
    if not ln2_triv:
        d["ln2C"] = nc.dram_tensor("ln2C", [P, E], F32, kind="ExternalInput")
    if not ln3_triv:
        d["ln3C"] = nc.dram_tensor("ln3C", [P, E], F32, kind="ExternalInput")
    out_h = nc.dram_tensor("out", [S, E], F32, kind="ExternalOutput")

    with tile.TileContext(nc) as tc:
        _emit(nc, tc, d, out_h, flags)
    _strip_self_waits(nc)
    _legalize_wait_counts(nc)
    return pr


def _ap3(t, off, d1, n1, d2, n2):
    """3D free AP over tile t: [partitions, (stride d1 x n1), (stride d2 x n2)]."""
    return bass.AP(tensor=t.tensor, offset=t.offset + off,
                   ap=[list(t.ap[0]), [d1, n1], [d2, n2]])


def _emit(nc, tc, d, out_h, flags):
    from contextlib import ExitStack
    gn1_triv, gn2_triv, ln2_triv, ln3_triv = flags
    import os
    tap = os.environ.get("KTAP", "")
    skips = set(os.environ.get("KSKIP", "").split(","))
    ctx = ExitStack()
    with ctx:
        p_c = ctx.enter_context(tc.tile_pool(name="const", bufs=1))
        p_w = ctx.enter_context(tc.tile_pool(name="w", bufs=2))
        p_w8 = ctx.enter_context(tc.tile_pool(name="w8", bufs=2))
        p_ld = ctx.enter_context(tc.tile_pool(name="ld", bufs=1))
        p_seq = ctx.enter_context(tc.tile_pool(name="seq", bufs=8))
        p_rot = ctx.enter_context(tc.tile_pool(name="rot", bufs=2))
        p_big = ctx.enter_context(tc.tile_pool(name="big", bufs=1))
        p_pair = ctx.enter_context(tc.tile_pool(name="pair", bufs=1))
        p_act = ctx.enter_context(tc.tile_pool(name="act", bufs=1))
        p_kv = ctx.enter_context(tc.tile_pool(name="kv", bufs=1))
        p_sc = ctx.enter_context(tc.tile_pool(name="scp", bufs=2))
        p_st = ctx.enter_context(tc.tile_pool(name="stp", bufs=1))
        p_sm = ctx.enter_context(tc.tile_pool(name="sm", bufs=4))
        p_res = ctx.enter_context(tc.tile_pool(name="res", bufs=2))
        pg = ctx.enter_context(tc.tile_pool(name="pg", bufs=2, space="PSUM"))
        psc = ctx.enter_context(tc.tile_pool(name="psc", bufs=2, space="PSUM"))
        pret = ctx.enter_context(tc.tile_pool(name="pret", bufs=2, space="PSUM"))
        pst = ctx.enter_context(tc.tile_pool(name="pst", bufs=1, space="PSUM"))
        pcro = ctx.enter_context(tc.tile_pool(name="pcro", bufs=1, space="PSUM"))

        # ---- consts ----
        cmask4 = p_c.tile([P, 4 * P], BF16)
        nc.sync.dma_start(out=cmask4, in_=d["cmask4"][:, :])
        eps_gn = p_c.tile([P, 1], F32)
        nc.vector.memset(eps_gn, 1e-5)
        eps_rms = p_c.tile([P, 1], F32)
        nc.vector.memset(eps_rms, 1e-6)
        gcons = {}
        for nm in ("gsb1", "gbb1", "gsb2", "gbb2", "ln2C", "ln3C"):
            if nm in d:
                t = p_c.tile([P, E], F32, name=nm)
                nc.sync.dma_start(out=t, in_=d[nm][:, :])
                gcons[nm] = t

        def load_wbf(nm):
            wt = p_w.tile([P, NF * E], BF16, tag=f"w{nm[1]}", name=nm)
            nc.sync.dma_start(
                out=wt, in_=d[nm][:, :].rearrange("(a p) e -> p a e", p=P))
            return wt

        def load_w8(nm, tag):
            wt = p_w8.tile([P, NF * E], FP8, tag=tag, name=nm)
            nc.sync.dma_start(out=wt, in_=d[nm][:, :])
            return wt

        def w8_rhs(wt, p):
            # rhs [128, 2, 512] for k-pair p
            return _ap3(wt, p * 2 * E, E, 2, 1, E)

        def w8_lhs(wt, p, m):
            # lhsT [128, 2, 128] for k-pair p, out col block m
            return _ap3(wt, p * 2 * E + m * P, E, 2, 1, P)

        def pair_lhs(pt_tile, p_idx, c):
            # activation pair tile [128, 2048]: k-tile i at cols i*1024
            return _ap3(pt_tile, c * P, S, 2, 1, P)

        def pair_rhs(pt_tile, nh):
            return _ap3(pt_tile, nh * 512, S, 2, 1, 512)

        def big_col(bt, k, c0, w):
            return bass.AP(tensor=bt.tensor, offset=bt.offset + k * S + c0,
                           ap=[list(bt.ap[0]), [1, w]])

        def bigT_ap(bt, c):
            # dma-transpose dest: [128, (S,4), (1,128)] at chunk col c
            return _ap3(bt, c * P, S, NF, 1, P)

        def grp(t, n=8, w=DH):
            return bass.AP(tensor=t.tensor, offset=t.offset,
                           ap=[list(t.ap[0]), [w, n], [1, w]])

        def bcast(t, n=8, w=DH):
            return bass.AP(tensor=t.tensor, offset=t.offset,
                           ap=[list(t.ap[0]), [1, n], [0, w]])

        # ---- input load + bf16 copy + dma-transpose ----
        def load_input(src_h, seq_tag, big_tag, pair_tag, q_alt):
            seqs = []
            big = p_big.tile([P, NF * S], BF16, tag=big_tag, name=big_tag)
            lds = []
            for hc in range(2):
                ld = p_ld.tile([P, 4 * E], F32, tag="ld", name=f"ld_{hc}")
                nc.sync.dma_start(
                    out=ld, in_=src_h[hc * 512:(hc + 1) * 512, :]
                    .rearrange("(a p) e -> p a e", p=P))
                lds.append(ld)
            for c in range(NC):
                sq = p_seq.tile([P, E], BF16, tag=seq_tag, name=f"{seq_tag}{c}")
                eng = nc.vector if c % 2 else nc.gpsimd
                eng.tensor_copy(
                    sq, lds[c // 4][:, (c % 4) * E:(c % 4 + 1) * E])
                seqs.append(sq)
                nc.scalar.dma_start_transpose(bigT_ap(big, c), sq)
            pairs = []
            for pi in range(2):
                pt = p_pair.tile([P, 2 * S], FP8, tag=f"{pair_tag}{pi}",
                                 name=f"{pair_tag}{pi}")
                for hf in range(2):
                    eng = nc.vector if (pi + hf) % 2 == 0 else nc.gpsimd
                    eng.tensor_copy(pt[:, hf * S:(hf + 1) * S],
                                    big[:, (pi * 2 + hf) * S:(pi * 2 + hf + 1) * S])
                pairs.append(pt)
            return seqs, big, pairs

        xb, xT, x8 = load_input(d["x"], "seqA", "bigA", "x8", 0)
        kqm_b = p_c.tile([P, NF * S], BF16, name="kqm_b")
        nc.sync.dma_start(
            out=kqm_b, in_=d["kqm"][:, :].rearrange("(a p) e -> p a e", p=P))
        kkm_b = p_c.tile([P, NF * S], BF16, name="kkm_b")
        nc.sync.dma_start(
            out=kkm_b, in_=d["kkm"][:, :].rearrange("(a p) e -> p a e", p=P))
        wq1t = load_wbf("wqc1"); wk1t = load_wbf("wkc1"); wv1t = load_wbf("wvc1")
        kp81 = load_w8("kp8_1", "kp8")
        ob, oT, o8 = load_input(d["obs"], "seqB", "bigB", "o8", 1)
        wg81 = load_w8("wg8_1", "wg8")
        wo81 = load_w8("wo8_1", "wo8")
        kksb = []
        for hc in range(2):
            kt = p_c.tile([P, 4 * E], BF16, name=f"kksb{hc}")
            nc.sync.dma_start(
                out=kt, in_=d["kks"][hc * 512:(hc + 1) * 512, :]
                .rearrange("(a p) e -> p a e", p=P))
            kksb.append(kt)
        wq2t = load_wbf("wqc2"); wk2t = load_wbf("wkc2"); wv2t = load_wbf("wvc2")
        kp82 = load_w8("kp8_2", "kp8"); wg82 = load_w8("wg8_2", "wg8")
        wo82 = load_w8("wo8_2", "wo8")
        fg8 = load_w8("fg8", "kp8"); fl8w = load_w8("fl8", "wg8")
        fo8 = load_w8("fo8", "wo8")

        def dump_seq(tiles, dt=F32):
            for c, t in enumerate(tiles[:NC]):
                o = p_ld.tile([P, E], F32, tag="dmp", name=f"dmp{c}")
                nc.vector.tensor_copy(o, t[:, :E])
                nc.sync.dma_start(out=out_h[c * P:(c + 1) * P, :], in_=o)

        def dump_fm(tiles, width=S):
            # feature-major tiles [128, width] -> out rows
            for k, t in enumerate(tiles[:NF]):
                o = p_ld.tile([P, S], F32, tag="dmpf", name=f"dmpf{k}", bufs=2)
                nc.vector.tensor_copy(o[:, :width], t[:, :width])
                rows = width // E
                for rr in range(rows):
                    nc.sync.dma_start(
                        out=out_h[(k * rows + rr) * P:(k * rows + rr + 1) * P, :],
                        in_=o[:, rr * E:(rr + 1) * E])

        # ================= MSR =================
        def msr(idx, qT_big, q8_pairs, kvT_big, kv8_pairs, resid_seq, wq_t, wk_t,
                wv_t, kp8, wg8, wo8, gn_triv, gsb, gbb, out_seq_tag, tap=""):
            sfx = str(idx)
            # --- q~ / k~ feature-major gemms (bf16) + decay-map evac ---
            qs = []
            ks = []
            for pt in range(NF):
                qs.append(p_act.tile([P, S], BF16, tag=f"qs{pt}", name=f"qs{sfx}_{pt}"))
                ks.append(p_act.tile([P, S], BF16, tag=f"ks{pt}", name=f"ks{sfx}_{pt}"))
            for pt in range(NF):
                for nh in range(2):
                    ps = pg.tile([P, 512], F32, tag="pg", name=f"q_{pt}_{nh}")
                    for k in range(NF):
                        nc.tensor.matmul(ps, wq_t[:, k * E + pt * P:
                                                   k * E + (pt + 1) * P],
                                         big_col(qT_big, k, nh * 512, 512),
                                         start=(k == 0), stop=(k == NF - 1))
                    nc.vector.tensor_mul(
                        qs[pt][:, nh * 512:(nh + 1) * 512], ps,
                        kqm_b[:, pt * S + nh * 512:pt * S + (nh + 1) * 512])
            for pt in range(NF):
                for nh in range(2):
                    ps = pg.tile([P, 512], F32, tag="pg", name=f"k_{pt}_{nh}")
                    for k in range(NF):
                        nc.tensor.matmul(ps, wk_t[:, k * E + pt * P:
                                                   k * E + (pt + 1) * P],
                                         big_col(kvT_big, k, nh * 512, 512),
                                         start=(k == 0), stop=(k == NF - 1))
                    nc.vector.tensor_mul(
                        ks[pt][:, nh * 512:(nh + 1) * 512], ps,
                        kkm_b[:, pt * S + nh * 512:pt * S + (nh + 1) * 512])
            if tap == "qs":
                dump_fm(qs)
                return None
            if tap == "ks":
                dump_fm(ks)
                return None

            # --- V (bf16) and K_seq (fp8 DR) seq-major gemms ---
            V = []
            Kq = []
            for c in range(NC):
                ps = pg.tile([P, 512], F32, tag="pg", name=f"v_{c}")
                for k in range(NF):
                    nc.tensor.matmul(ps, big_col(kvT_big, k, c * P, P),
                                     wv_t[:, k * E:(k + 1) * E],
                                     start=(k == 0), stop=(k == NF - 1))
                vt = p_kv.tile([P, 512], BF16, tag=f"V{c}", name=f"V{sfx}_{c}")
                nc.scalar.copy(vt, ps)
                V.append(vt)
            for c in range(NC):
                ps = pg.tile([P, 512], F32, tag="pg", name=f"kq_{c}")
                for pi in range(2):
                    nc.tensor.matmul(ps, pair_lhs(kv8_pairs[pi], pi, c),
                                     w8_rhs(kp8, pi), start=(pi == 0),
                                     stop=(pi == 1), perf_mode=DRM)
                kt = p_kv.tile([P, 512], BF16, tag=f"K{c}", name=f"K{sfx}_{c}")
                nc.vector.tensor_mul(
                    kt, ps, kksb[c // 4][:, (c % 4) * E:(c % 4 + 1) * E])
                Kq.append(kt)
            if tap == "V":
                dump_seq(V)
                return None
            if tap == "Kq":
                dump_seq(Kq)
                return None

            # --- gate gemm (fp8 DR, feature-major) + silu ---
            gfm = [p_act.tile([P, S], BF16, tag=f"gf{m}", name=f"gfm{sfx}_{m}")
                   for m in range(NF)]
            for m in ([] if "gate" in skips else range(NF)):
                for nh in range(2):
                    ps = pg.tile([P, 512], F32, tag="pg", name=f"g_{m}_{nh}")
                    for pi in range(2):
                        nc.tensor.matmul(ps, w8_lhs(wg8, pi, m),
                                         pair_rhs(q8_pairs[pi], nh),
                                         start=(pi == 0), stop=(pi == 1),
                                         perf_mode=DRM)
                    nc.scalar.activation(gfm[m][:, nh * 512:(nh + 1) * 512], ps,
                                         AF.Silu)

            # --- retention chunks (software-pipelined, batched GN stats) ---
            stc = []
            rn = []
            ret_sb = []
            sc_all = []
            sums_all = p_sm.tile([P, 64], F32, tag="sumsA", name=f"sumsA{sfx}",
                                 bufs=2)
            sqs_all = p_sm.tile([P, 64], F32, tag="sqsA", name=f"sqsA{sfx}",
                                bufs=2)

            def emit_scores(c):
                # bank parity = head parity so every matmul in a bank shares
                # the same tile_position row base (mixing row bases in one
                # bank breaks execution)
                sc_sb = []
                for par in range(2):
                    ps = psc.tile([P, 512], F32, tag="psc", name=f"sc_{c}_{par}")
                    sl = par * DH
                    for hh in range(4):
                        h = 2 * hh + par
                        ptt = h // 2
                        nc.tensor.matmul(
                            ps[:, hh * P:(hh + 1) * P],
                            ks[ptt][sl:sl + DH, c * P:(c + 1) * P],
                            qs[ptt][sl:sl + DH, c * P:(c + 1) * P],
                            start=(hh == 0), stop=(hh == 3),
                            skip_group_check=True)
                    sb = p_sc.tile([P, 512], BF16, tag=f"scsb{par}",
                                   name=f"scsb_{c}_{par}")
                    nc.vector.tensor_mul(sb, ps, cmask4)
                    sc_sb.append(sb)
                sc_all.append(sc_sb)

            def emit_stage(c):
                # state update first so chunk c+1's cross input is ready early
                if c < NC - 1 and "state" not in skips:
                    pstt = pst.tile([P, 256], F32, tag="pst", name=f"st_{c}",
                                    bufs=1)
                    for h in range(H):
                        ptt, sl = h // 2, (h % 2) * DH
                        nc.tensor.matmul(
                            pstt[sl:sl + DH, ptt * DH:(ptt + 1) * DH],
                            Kq[c][:, h * DH:(h + 1) * DH],
                            V[c][:, h * DH:(h + 1) * DH],
                            start=(h < 2), stop=(h >= H - 2),
                            skip_group_check=True, tile_position=(0, sl))
                    st = p_sm.tile([P, 256], BF16, tag="stc", name=f"stc_{c}",
                                   bufs=2)
                    if c == 0:
                        nc.vector.tensor_copy(st, pstt)
                    else:
                        nc.vector.tensor_add(st, pstt, stc[c - 1])
                    stc.append(st)
                prt = pret.tile([P, 512], F32, tag="pret", name=f"ret_{c}")
                cross = (c > 0) and ("state" not in skips) and \
                    ("intra" not in skips)
                for h in ([] if "intra" in skips else range(H)):
                    nc.tensor.matmul(
                        prt[:, h * DH:(h + 1) * DH],
                        sc_all[c][h % 2][:, (h // 2) * P:(h // 2 + 1) * P],
                        V[c][:, h * DH:(h + 1) * DH],
                        start=(h == 0), stop=(h == H - 1 and not cross),
                        skip_group_check=True)
                if cross:
                    for h in range(0, H, 2):       # even heads: row base 0
                        ptt = h // 2
                        nc.tensor.matmul(
                            prt[:, h * DH:(h + 1) * DH],
                            qs[ptt][0:DH, c * P:(c + 1) * P],
                            stc[c - 1][0:DH, ptt * DH:(ptt + 1) * DH],
                            start=False, stop=(h == H - 2),
                            skip_group_check=True)
                    pco = pcro.tile([P, 256], F32, tag="pcro", name=f"cro_{c}")
                    for h in range(1, H, 2):       # odd heads: row base 64
                        ptt = h // 2
                        nc.tensor.matmul(
                            pco[:, ptt * DH:(ptt + 1) * DH],
                            qs[ptt][DH:2 * DH, c * P:(c + 1) * P],
                            stc[c - 1][DH:2 * DH, ptt * DH:(ptt + 1) * DH],
                            start=(h == 1), stop=(h == H - 1),
                            skip_group_check=True)
                rb = p_seq.tile([P, 512], BF16, tag="retsb", name=f"retsb_{c}")
                if "intra" in skips:
                    nc.vector.memset(rb, 0.0)
                else:
                    nc.scalar.copy(rb, prt)
                    if cross:
                        def _odd(t, w):
                            return bass.AP(tensor=t.tensor,
                                           offset=t.offset + (DH if w else 0),
                                           ap=[list(t.ap[0]),
                                               [2 * DH if w else DH, 4],
                                               [1, DH]])
                        nc.vector.tensor_add(_odd(rb, True), _odd(rb, True),
                                             _odd(pco, False))
                ret_sb.append(rb)
                if "gn" not in skips:
                    sq = p_rot.tile([P, 512], BF16, tag="gnsq",
                                    name=f"gnsq_{c}")
                    nc.scalar.activation(sq, rb, AF.Square)
                    nc.vector.tensor_reduce(sums_all[:, c * H:(c + 1) * H],
                                            grp(rb),
                                            axis=mybir.AxisListType.X,
                                            op=ALU.add)
                    nc.vector.tensor_reduce(sqs_all[:, c * H:(c + 1) * H],
                                            grp(sq),
                                            axis=mybir.AxisListType.X,
                                            op=ALU.add)

            # narrow GN stats in two chunk-halves so the first half's
            # applies + rnT transposes overlap the second half's retention
            mu = p_sm.tile([P, 64], F32, tag="mu", name=f"mu{sfx}", bufs=1)
            m2 = p_sm.tile([P, 64], F32, tag="m2", name=f"m2{sfx}", bufs=1)
            msq = p_sm.tile([P, 64], F32, tag="msq", name=f"msq{sfx}", bufs=1)
            var = p_sm.tile([P, 64], F32, tag="var", name=f"var{sfx}", bufs=1)
            sd = p_sm.tile([P, 64], F32, tag="sd", name=f"sd{sfx}", bufs=1)
            rs = p_sm.tile([P, 64], F32, tag="rs", name=f"rs{sfx}", bufs=1)
            c2 = p_sm.tile([P, 64], F32, tag="c2", name=f"c2{sfx}", bufs=1)

            def bc8(t, c):
                return bass.AP(tensor=t.tensor, offset=t.offset + c * H,
                               ap=[list(t.ap[0]), [1, H], [0, DH]])

            def stats_and_apply(hf):
                hs = slice(hf * 32, hf * 32 + 32)
                nc.vector.tensor_scalar_mul(mu[:, hs], sums_all[:, hs],
                                            1.0 / DH)
                nc.gpsimd.tensor_mul(m2[:, hs], mu[:, hs], mu[:, hs])
                nc.gpsimd.tensor_scalar_mul(msq[:, hs], sqs_all[:, hs],
                                            1.0 / DH)
                nc.vector.tensor_sub(var[:, hs], msq[:, hs], m2[:, hs])
                nc.scalar.activation(sd[:, hs], var[:, hs], AF.Sqrt,
                                     bias=eps_gn)
                nc.vector.reciprocal(rs[:, hs], sd[:, hs])
                nc.gpsimd.tensor_mul(c2[:, hs], mu[:, hs], rs[:, hs])
                for c in range(hf * 4, hf * 4 + 4):
                    tmp = p_rot.tile([P, 512], BF16, tag="gntmp",
                                     name=f"gntmp_{c}")
                    nc.gpsimd.tensor_mul(grp(tmp), grp(ret_sb[c]), bc8(rs, c))
                    rt = p_rot.tile([P, 512], BF16, tag="rn", name=f"rn_{c}",
                                    bufs=2)
                    nc.gpsimd.tensor_sub(grp(rt), grp(tmp), bc8(c2, c))
                    if not gn_triv:
                        nc.vector.tensor_mul(rt, rt, gsb)
                        nc.vector.tensor_add(rt, rt, gbb)
                    rn.append(rt)
                    if tap != "rnnt":
                        nc.scalar.dma_start_transpose(bigT_ap(rnT, c), rt)

            import os as _os2
            if _os2.environ.get("KNOSPLIT"):
                emit_scores(0)
                for c in range(1, NC):
                    emit_scores(c)
                    emit_stage(c - 1)
                emit_stage(NC - 1)
                if tap == "ret":
                    dump_seq(ret_sb)
                    return None
                stats_and_apply(0)
                stats_and_apply(1)
            else:
                emit_scores(0)
                for c in range(1, NC):
                    emit_scores(c)
                    emit_stage(c - 1)
                    if c == 5:
                        stats_and_apply(0)
                emit_stage(NC - 1)
                if tap == "ret":
                    dump_seq(ret_sb)
                    return None
                stats_and_apply(1)
            if tap in ("rn", "rnnt"):
                dump_seq(rn)
                return None

            # --- gating (feature-major) -> fp8 pair tiles ---
            g8 = [p_pair.tile([P, 2 * S], FP8, tag=f"gated{pi}",
                              name=f"gated{sfx}_{pi}") for pi in range(2)]
            for m in range(NF):
                nc.gpsimd.tensor_mul(
                    g8[m // 2][:, (m % 2) * S:(m % 2 + 1) * S],
                    gfm[m], rnT[:, m * S:(m + 1) * S])

            # --- W_O gemm (fp8 DR, seq-major out) + residual + RMSNorm ---
            outs = []
            for c in range(NC):
                ps = pg.tile([P, 512], F32, tag="pg", name=f"wo_{c}")
                for pi in range(2):
                    nc.tensor.matmul(ps, pair_lhs(g8[pi], pi, c),
                                     w8_rhs(wo8, pi), start=(pi == 0),
                                     stop=(pi == 1), perf_mode=DRM)
                res = p_res.tile([P, E], F32, tag="res", name=f"res{sfx}_{c}")
                nc.vector.tensor_add(res, ps, resid_seq[c])
                ssq = p_sm.tile([P, 1], F32, tag="ssq", name=f"ssq_{c}", bufs=2)
                ts = p_rot.tile([P, E], BF16, tag="gnsq", name=f"ttr_{c}")
                nc.scalar.activation(ts, res, AF.Square)
                nc.vector.tensor_reduce(ssq, ts, axis=mybir.AxisListType.X,
                                        op=ALU.add)
                sdr = p_sm.tile([P, 1], F32, tag="sdr", name=f"sdr_{c}", bufs=2)
                nc.scalar.activation(sdr, ssq, AF.Sqrt, bias=eps_rms,
                                     scale=1.0 / E)
                rsr = p_sm.tile([P, 1], F32, tag="rsr", name=f"rsr_{c}", bufs=2)
                nc.vector.reciprocal(rsr, sdr)
                o = p_seq.tile([P, E], BF16, tag=out_seq_tag,
                               name=f"{out_seq_tag}{c}")
                nc.scalar.activation(o, res, AF.Identity, scale=rsr)
                outs.append(o)
            return outs

        # rnT big tile shared by both msrs (rotates)
        rnT = p_big.tile([P, NF * S], BF16, tag="bigR", name="rnT1")
        r = msr(1, xT, x8, xT, x8, xb, wq1t, wk1t,
                wv1t, kp81, wg81, wo81,
                gn1_triv, gcons.get("gsb1"), gcons.get("gbb1"), "seqA",
                tap=tap if tap in ("qs", "ks", "V", "Kq", "rn", "ret", "rnnt") else "")
        if tap in ("qs", "ks", "V", "Kq", "rn", "ret", "rnnt"):
            return
        x1 = r
        if tap == "x1":
            dump_seq(x1)
            return

        # x1 -> feature-major (dma transpose) + fp8 pairs
        x1T = p_big.tile([P, NF * S], BF16, tag="bigA", name="x1T")
        for c in range(NC):
            nc.scalar.dma_start_transpose(bigT_ap(x1T, c), x1[c])
        x18 = []
        for pi in range(2):
            pt = p_pair.tile([P, 2 * S], FP8, tag=f"x8{pi}", name=f"x18_{pi}")
            for hf in range(2):
                eng = nc.vector if (pi + hf) % 2 == 0 else nc.gpsimd
                eng.tensor_copy(pt[:, hf * S:(hf + 1) * S],
                                x1T[:, (pi * 2 + hf) * S:(pi * 2 + hf + 1) * S])
            x18.append(pt)

        rnT = p_big.tile([P, NF * S], BF16, tag="bigR", name="rnT2")
        r = msr(2, oT, o8, x1T, x18, ob, wq2t, wk2t,
                wv2t, kp82, wg82, wo82,
                gn2_triv, gcons.get("gsb2"), gcons.get("gbb2"), "seqC")
        x2 = r
        if not ln2_triv:
            for c in range(NC):
                nc.gpsimd.tensor_mul(x2[c], x2[c], gcons["ln2C"])
        if tap == "x2":
            dump_seq(x2)
            return

        # x2 -> feature-major + fp8 pairs (ffn inputs)
        x2T = p_big.tile([P, NF * S], BF16, tag="bigR", name="x2T")
        for c in range(NC):
            nc.scalar.dma_start_transpose(bigT_ap(x2T, c), x2[c])
        x28 = []
        for pi in range(2):
            pt = p_pair.tile([P, 2 * S], FP8, tag=f"o8{pi}", name=f"x28_{pi}")
            for hf in range(2):
                eng = nc.vector if (pi + hf) % 2 == 0 else nc.gpsimd
                eng.tensor_copy(pt[:, hf * S:(hf + 1) * S],
                                x2T[:, (pi * 2 + hf) * S:(pi * 2 + hf + 1) * S])
            x28.append(pt)

        # ---- FFN (all fp8 DR) ----
        ffg = [p_act.tile([P, S], BF16, tag=f"qs{m}", name=f"ffg_{m}")
               for m in range(NF)]
        ffl = [p_act.tile([P, S], BF16, tag=f"ks{m}", name=f"ffl_{m}")
               for m in range(NF)]
        for m in range(NF):
            for nh in range(2):
                ps = pg.tile([P, 512], F32, tag="pg", name=f"fg_{m}_{nh}")
                for pi in range(2):
                    nc.tensor.matmul(ps, w8_lhs(fg8, pi, m),
                                     pair_rhs(x28[pi], nh), start=(pi == 0),
                                     stop=(pi == 1), perf_mode=DRM)
                nc.scalar.activation(ffg[m][:, nh * 512:(nh + 1) * 512], ps,
                                     AF.Silu)
        for m in range(NF):
            for nh in range(2):
                ps = pg.tile([P, 512], F32, tag="pg", name=f"fl_{m}_{nh}")
                for pi in range(2):
                    nc.tensor.matmul(ps, w8_lhs(fl8w, pi, m),
                                     pair_rhs(x28[pi], nh), start=(pi == 0),
                                     stop=(pi == 1), perf_mode=DRM)
                nc.vector.tensor_copy(ffl[m][:, nh * 512:(nh + 1) * 512], ps)
        fl8t = [p_pair.tile([P, 2 * S], FP8, tag=f"gated{pi}", name=f"flT8_{pi}")
                for pi in range(2)]
        for m in range(NF):
            nc.gpsimd.tensor_mul(fl8t[m // 2][:, (m % 2) * S:(m % 2 + 1) * S],
                                 ffg[m], ffl[m])
        for c in range(NC):
            ps = pg.tile([P, 512], F32, tag="pg", name=f"fo_{c}")
            for pi in range(2):
                nc.tensor.matmul(ps, pair_lhs(fl8t[pi], pi, c),
                                 w8_rhs(fo8, pi), start=(pi == 0),
                                 stop=(pi == 1), perf_mode=DRM)
            res = p_res.tile([P, E], F32, tag="res", name=f"res3_{c}")
            nc.vector.tensor_add(res, ps, x2[c])
            ssq = p_sm.tile([P, 1], F32, tag="ssq", name=f"ssq3_{c}", bufs=2)
            ts = p_rot.tile([P, E], BF16, tag="gnsq", name=f"ttr3_{c}")
            nc.scalar.activation(ts, res, AF.Square)
            nc.vector.tensor_reduce(ssq, ts, axis=mybir.AxisListType.X,
                                    op=ALU.add)
            sdr = p_sm.tile([P, 1], F32, tag="sdr", name=f"sdr3_{c}", bufs=2)
            nc.scalar.activation(sdr, ssq, AF.Sqrt, bias=eps_rms, scale=1.0 / E)
            rsr = p_sm.tile([P, 1], F32, tag="rsr", name=f"rsr3_{c}", bufs=2)
            nc.vector.reciprocal(rsr, sdr)
            if c % 4 == 0:
                obig = p_ld.tile([P, 4 * E], F32, tag="oo", name=f"oo_{c // 4}",
                                 bufs=1)
            o = obig[:, (c % 4) * E:(c % 4 + 1) * E]
            nc.scalar.activation(o, res, AF.Identity, scale=rsr)
            if not ln3_triv:
                nc.gpsimd.tensor_mul(o, o, gcons["ln3C"])
            if c % 4 == 3:
                nc.sync.dma_start(
                    out=out_h[(c - 3) * P:(c + 1) * P, :]
                    .rearrange("(a p) e -> p a e", p=P), in_=obig)


_prog_cache = {}


def _get_program(flags=(True, True, True, True)):
    if flags not in _prog_cache:
        _prog_cache[flags] = _build_program(flags)
    return _prog_cache[flags]


def kernel(**inputs):
    inputs = {k: np.asarray(v) for k, v in inputs.items()}
    flags = _flags(inputs)
    consts = _build_consts(inputs)
    pr = _get_program(flags)
    x = np.ascontiguousarray(inputs["x"], dtype=np.float32)
    obs = np.ascontiguousarray(inputs["obs_rep"], dtype=np.float32)
    in_maps = []
    for b in range(N_CORES):
        m = dict(consts)
        m["x"] = np.ascontiguousarray(x[b])
        m["obs"] = np.ascontiguousarray(obs[b])
        in_maps.append(m)
    res = run_bass_kernel_spmd(pr.nc, in_maps, core_ids=list(range(N_CORES)))
    return np.stack([res.results[b]["out"] for b in range(N_CORES)], axis=0)


# revision 4
# speedup vs baseline: 1.1942x; 1.0618x over previous
"""Trainium2 Bass kernel for nn_DecodeBlock (RetNet-style decoder block), v2.

Sharding: data-parallel over batch (B=8) across 8 NeuronCores; no collectives.

Design (per core, vs the quadratic v1 baseline):
  - Chunked-recurrent retention (C=128): per chunk, intra-chunk scores
    [128,128] + cross-chunk contribution through a per-head [dk,dv] state
    accumulated in PSUM across chunks (global kappa^±n scaling keeps the
    recurrence a pure sum; exact, no approximation).
  - fp8-e4m3 DoubleRow matmuls (2 k-tiles per pass, 0.5 cyc/row) for the
    K_seq/gate/W_O/FFN gemms; q/k/V projections stay bf16 (precision).
  - Sequence-major normalization path: GroupNorm/RMSNorm stats as [128,8]
    narrow tiles (engine cost scales with free-size), ACT per-partition
    Rsqrt/scale application, residuals fused into gemm evacuations.
  - All transposes via the DMA XBAR (dma_start_transpose, bf16), not PE.
  - Output is produced sequence-major and DMA'd straight out.
"""

import numpy as np

import concourse.bass as bass
import concourse.mybir as mybir
import concourse.tile as tile
from concourse.bass_utils import run_bass_kernel_spmd

F32 = mybir.dt.float32
BF16 = mybir.dt.bfloat16
FP8 = mybir.dt.float8e4
AF = mybir.ActivationFunctionType
ALU = mybir.AluOpType
DRM = mybir.MatmulPerfMode.DoubleRow

E, H, B, S = 512, 8, 8, 1024
DH = E // H          # 64
P = 128
NF = E // P          # 4 feature tiles
NC = S // P          # 8 seq chunks

N_CORES = 8


def _kappas():
    k = 1.0 - np.exp(np.linspace(np.log(1.0 / 32.0), np.log(1.0 / 512.0), H))
    return k.astype(np.float64)


def _pair8(w):
    """[E, E] weight -> fp8 DR layout [128, 4*512]: col block j*512 = k-tile j
    (rows j*128..j*128+127)."""
    import ml_dtypes
    w = np.asarray(w, np.float32)
    return np.ascontiguousarray(
        w.reshape(NF, P, E).transpose(1, 0, 2).reshape(P, NF * E)
        .astype(ml_dtypes.float8_e4m3))


def _build_consts(inputs):
    import ml_dtypes
    bf16 = ml_dtypes.bfloat16
    kap = _kappas()
    n = np.arange(S, dtype=np.float64)
    kq = np.empty((E, S), np.float64)
    kk = np.empty((E, S), np.float64)
    for h in range(H):
        kq[h * DH:(h + 1) * DH, :] = (kap[h] ** n)[None, :]
        kk[h * DH:(h + 1) * DH, :] = (kap[h] ** (-n))[None, :]
    kks = np.empty((S, E), np.float64)   # seq-major kappa^-m, head-major cols
    for h in range(H):
        kks[:, h * DH:(h + 1) * DH] = (kap[h] ** (-n))[:, None]
    # causal keep n>=m, [128,128] repeated 4x along free
    cm = (np.arange(P)[None, :] >= np.arange(P)[:, None]).astype(np.float32)
    cmask4 = np.ascontiguousarray(np.tile(cm, (1, 4)).astype(bf16))

    ln1 = np.asarray(inputs["ln1_s"], np.float32)
    ln2 = np.asarray(inputs["ln2_s"], np.float32)

    def conc(w):
        return np.asarray(w, np.float32).transpose(1, 0, 2).reshape(E, E)

    wq1 = conc(inputs["wq1"]); wk1 = conc(inputs["wk1"]); wv1 = conc(inputs["wv1"])
    wq2 = conc(inputs["wq2"])
    wk2f = ln1[:, None] * conc(inputs["wk2"])   # fold ln1 into msr2 kv path
    wv2f = ln1[:, None] * conc(inputs["wv2"])
    fgf = ln2[:, None] * np.asarray(inputs["ffn_w_gate"], np.float32)
    flf = ln2[:, None] * np.asarray(inputs["ffn_w_lin"], np.float32)

    consts = {
        "kqm": np.ascontiguousarray(kq.astype(bf16)),
        "kkm": np.ascontiguousarray(kk.astype(bf16)),
        "kks": np.ascontiguousarray(kks.astype(bf16)),
        "cmask4": cmask4,
        "wqc1": np.ascontiguousarray(wq1.astype(bf16)),
        "wkc1": np.ascontiguousarray(wk1.astype(bf16)),
        "wvc1": np.ascontiguousarray(wv1.astype(bf16)),
        "wqc2": np.ascontiguousarray(wq2.astype(bf16)),
        "wkc2": np.ascontiguousarray(wk2f.astype(bf16)),
        "wvc2": np.ascontiguousarray(wv2f.astype(bf16)),
        "kp8_1": _pair8(wk1), "kp8_2": _pair8(wk2f),
        "wg8_1": _pair8(inputs["wg1"]), "wg8_2": _pair8(inputs["wg2"]),
        "wo8_1": _pair8(inputs["wo1"]), "wo8_2": _pair8(inputs["wo2"]),
        "fg8": _pair8(fgf), "fl8": _pair8(flf),
        "fo8": _pair8(inputs["ffn_w_out"]),
    }
    fl = _flags(inputs)
    if not fl[0]:
        consts["gsb1"] = np.ascontiguousarray(
            np.tile(np.asarray(inputs["gs1"], np.float32), (P, 1)))
        consts["gbb1"] = np.ascontiguousarray(
            np.tile(np.asarray(inputs["gb1"], np.float32), (P, 1)))
    if not fl[1]:
        consts["gsb2"] = np.ascontiguousarray(
            np.tile(np.asarray(inputs["gs2"], np.float32), (P, 1)))
        consts["gbb2"] = np.ascontiguousarray(
            np.tile(np.asarray(inputs["gb2"], np.float32), (P, 1)))
    if not fl[2]:
        consts["ln2C"] = np.ascontiguousarray(np.tile(ln2, (P, 1)))
    if not fl[3]:
        consts["ln3C"] = np.ascontiguousarray(
            np.tile(np.asarray(inputs["ln3_s"], np.float32), (P, 1)))
    return consts


def _flags(inputs):
    """(gn1 trivial, gn2 trivial, ln2 trivial, ln3 trivial)"""
    return (
        bool(np.allclose(inputs["gs1"], 1) and np.allclose(inputs["gb1"], 0)),
        bool(np.allclose(inputs["gs2"], 1) and np.allclose(inputs["gb2"], 0)),
        bool(np.allclose(inputs["ln2_s"], 1)),
        bool(np.allclose(inputs["ln3_s"], 1)),
    )


class _Prog:
    pass


def _strip_self_waits(nc):
    import concourse.mybir as mb
    for f in nc.m.functions:
        for blk in f.blocks:
            for inst in blk.instructions:
                si = getattr(inst, "sync_info", None)
                if si is None or not si.on_wait:
                    continue
                tname = type(inst).__name__
                if tname in ("InstDMACopy", "InstDrain", "InstEventSemaphore",
                             "InstTriggerDma", "InstDmaTransposeAnt"):
                    continue
                eng = getattr(inst, "engine", None)
                eng_name = getattr(eng, "name", str(eng))
                pref = {"PE": "PE_", "DVE": "DVE_", "Activation": "Activation_",
                        "Pool": "Pool_", "SP": "SP_"}.get(eng_name)
                if not pref:
                    continue
                kept = [w for w in si.on_wait if not str(w.ant_name).startswith(pref)]
                if len(kept) != len(si.on_wait):
                    si.on_wait = kept


_MAX_WAITS = 1
_WAIT_BUDGET = {"InstActivation": 1, "InstDrain": 0}


def _legalize_wait_counts(nc):
    import bass_rust
    import concourse.mybir as mb
    uid = [0]
    for f in nc.m.functions:
        for blk in f.blocks:
            insts = list(blk.instructions)
            out = []
            changed = False
            for inst in insts:
                si = getattr(inst, "sync_info", None)
                waits = list(si.on_wait) if si and si.on_wait else []
                plain = [w for w in waits if w.sync_type == "semaphore"]
                other = [w for w in waits if w.sync_type != "semaphore"]
                cap = _WAIT_BUDGET.get(type(inst).__name__, _MAX_WAITS)
                if len(plain) + len(other) > cap and len(plain) > 0:
                    budget = max(0, cap - len(other))
                    keep, excess = plain[:budget], plain[budget:]
                    while excess:
                        chunk, excess = excess[:1], excess[1:]
                        nop = bass_rust.InstNoOp(name=f"wnop-{uid[0]}", ins=[], outs=[])
                        uid[0] += 1
                        nop.engine = inst.engine
                        nop.sync_info = mb.SyncInfo(on_wait=chunk, on_update=[])
                        out.append(nop)
                    si.on_wait = other + keep
                    changed = True
                out.append(inst)
            if changed:
                blk.instructions = out


def _build_program(flags):
    nc = bass.Bass()
    pr = _Prog()
    pr.nc = nc
    d = {}
    d["x"] = nc.dram_tensor("x", [S, E], F32, kind="ExternalInput")
    d["obs"] = nc.dram_tensor("obs", [S, E], F32, kind="ExternalInput")
    for nm in ("wqc1", "wkc1", "wvc1", "wqc2", "wkc2", "wvc2"):
        d[nm] = nc.dram_tensor(nm, [E, E], BF16, kind="ExternalInput")
    for nm in ("kp8_1", "kp8_2", "wg8_1", "wg8_2", "wo8_1", "wo8_2",
               "fg8", "fl8", "fo8"):
        d[nm] = nc.dram_tensor(nm, [P, NF * E], FP8, kind="ExternalInput")
    d["kqm"] = nc.dram_tensor("kqm", [E, S], BF16, kind="ExternalInput")
    d["kkm"] = nc.dram_tensor("kkm", [E, S], BF16, kind="ExternalInput")
    d["kks"] = nc.dram_tensor("kks", [S, E], BF16, kind="ExternalInput")
    d["cmask4"] = nc.dram_tensor("cmask4", [P, 4 * P], BF16, kind="ExternalInput")
    gn1_triv, gn2_triv, ln2_triv, ln3_triv = flags
    if not gn1_triv:
        d["gsb1"] = nc.dram_tensor("gsb1", [P, E], F32, kind="ExternalInput")
        d["gbb1"] = nc.dram_tensor("gbb1", [P, E], F32, kind="ExternalInput")
    if not gn2_triv:
        d["gsb2"] = nc.dram_tensor("gsb2", [P, E], F32, kind="ExternalInput")
        d["gbb2"] = nc.dram_tensor("gbb2", [P, E], F32, kind="ExternalInput")
    if not ln2_triv:
        d["ln2C"] = nc.dram_tensor("ln2C", [P, E], F32, kind="ExternalInput")
    if not ln3_triv:
        d["ln3C"] = nc.dram_tensor("ln3C", [P, E], F32, kind="ExternalInput")
    out_h = nc.dram_tensor("out", [S, E], F32, kind="ExternalOutput")

    with tile.TileContext(nc) as tc:
        _emit(nc, tc, d, out_h, flags)
    _strip_self_waits(nc)
    _legalize_wait_counts(nc)
    return pr


def _ap3(t, off, d1, n1, d2, n2):
    """3D free AP over tile t: [partitions, (stride d1 x n1), (stride d2 x n2)]."""
    return bass.AP(tensor=t.tensor, offset=t.offset + off,
                   ap=[list(t.ap[0]), [d1, n1], [d2, n2]])


def _emit(nc, tc, d, out_h, flags):
    from contextlib import ExitStack
    gn1_triv, gn2_triv, ln2_triv, ln3_triv = flags
    import os
    tap = os.environ.get("KTAP", "")
    skips = set(os.environ.get("KSKIP", "").split(","))
    ctx = ExitStack()
    with ctx:
        p_c = ctx.enter_context(tc.tile_pool(name="const", bufs=1))
        p_w = ctx.enter_context(tc.tile_pool(name="w", bufs=2))
        p_w8 = ctx.enter_context(tc.tile_pool(name="w8", bufs=2))
        p_ld = ctx.enter_context(tc.tile_pool(name="ld", bufs=1))
        p_seq = ctx.enter_context(tc.tile_pool(name="seq", bufs=8))
        p_rot = ctx.enter_context(tc.tile_pool(name="rot", bufs=2))
        p_big = ctx.enter_context(tc.tile_pool(name="big", bufs=1))
        p_pair = ctx.enter_context(tc.tile_pool(name="pair", bufs=1))
        p_act = ctx.enter_context(tc.tile_pool(name="act", bufs=1))
        p_kv = ctx.enter_context(tc.tile_pool(name="kv", bufs=1))
        p_sc = ctx.enter_context(tc.tile_pool(name="scp", bufs=2))
        p_st = ctx.enter_context(tc.tile_pool(name="stp", bufs=1))
        p_sm = ctx.enter_context(tc.tile_pool(name="sm", bufs=4))
        p_res = ctx.enter_context(tc.tile_pool(name="res", bufs=2))
        pg = ctx.enter_context(tc.tile_pool(name="pg", bufs=2, space="PSUM"))
        psc = ctx.enter_context(tc.tile_pool(name="psc", bufs=2, space="PSUM"))
        pret = ctx.enter_context(tc.tile_pool(name="pret", bufs=2, space="PSUM"))
        pst = ctx.enter_context(tc.tile_pool(name="pst", bufs=1, space="PSUM"))
        pcro = ctx.enter_context(tc.tile_pool(name="pcro", bufs=1, space="PSUM"))

        # ---- consts ----
        cmask4 = p_c.tile([P, 4 * P], BF16)
        nc.sync.dma_start(out=cmask4, in_=d["cmask4"][:, :])
        eps_gn = p_c.tile([P, 1], F32)
        nc.vector.memset(eps_gn, 1e-5)
        eps_rms = p_c.tile([P, 1], F32)
        nc.vector.memset(eps_rms, 1e-6)
        gcons = {}
        for nm in ("gsb1", "gbb1", "gsb2", "gbb2", "ln2C", "ln3C"):
            if nm in d:
                t = p_c.tile([P, E], F32, name=nm)
                nc.sync.dma_start(out=t, in_=d[nm][:, :])
                gcons[nm] = t

        def load_wbf(nm):
            wt = p_w.tile([P, NF * E], BF16, tag=f"w{nm[1]}", name=nm)
            nc.sync.dma_start(
                out=wt, in_=d[nm][:, :].rearrange("(a p) e -> p a e", p=P))
            return wt

        def load_w8(nm, tag):
            wt = p_w8.tile([P, NF * E], FP8, tag=tag, name=nm)
            nc.sync.dma_start(out=wt, in_=d[nm][:, :])
            return wt

        def w8_rhs(wt, p):
            # rhs [128, 2, 512] for k-pair p
            return _ap3(wt, p * 2 * E, E, 2, 1, E)

        def w8_lhs(wt, p, m):
            # lhsT [128, 2, 128] for k-pair p, out col block m
            return _ap3(wt, p * 2 * E + m * P, E, 2, 1, P)

        def pair_lhs(pt_tile, p_idx, c):
            # activation pair tile [128, 2048]: k-tile i at cols i*1024
            return _ap3(pt_tile, c * P, S, 2, 1, P)

        def pair_rhs(pt_tile, nh):
            return _ap3(pt_tile, nh * 512, S, 2, 1, 512)

        def big_col(bt, k, c0, w):
            return bass.AP(tensor=bt.tensor, offset=bt.offset + k * S + c0,
                           ap=[list(bt.ap[0]), [1, w]])

        def bigT_ap(bt, c):
            # dma-transpose dest: [128, (S,4), (1,128)] at chunk col c
            return _ap3(bt, c * P, S, NF, 1, P)

        def grp(t, n=8, w=DH):
            return bass.AP(tensor=t.tensor, offset=t.offset,
                           ap=[list(t.ap[0]), [w, n], [1, w]])

        def bcast(t, n=8, w=DH):
            return bass.AP(tensor=t.tensor, offset=t.offset,
                           ap=[list(t.ap[0]), [1, n], [0, w]])

        # ---- input load + bf16 copy + dma-transpose ----
        def load_input(src_h, seq_tag, big_tag, pair_tag, q_alt):
            seqs = []
            big = p_big.tile([P, NF * S], BF16, tag=big_tag, name=big_tag)
            lds = []
            for qc in range(4):
                ld = p_ld.tile([P, 2 * E], F32, tag="ld", name=f"ld_{qc}",
                               bufs=2)
                nc.sync.dma_start(
                    out=ld, in_=src_h[qc * 256:(qc + 1) * 256, :]
                    .rearrange("(a p) e -> p a e", p=P))
                lds.append(ld)
            for c in range(NC):
                sq = p_seq.tile([P, E], BF16, tag=seq_tag, name=f"{seq_tag}{c}")
                eng = nc.vector if c % 2 else nc.gpsimd
                eng.tensor_copy(
                    sq, lds[c // 2][:, (c % 2) * E:(c % 2 + 1) * E])
                seqs.append(sq)
                qeng = nc.scalar if c % 2 else nc.sync
                qeng.dma_start_transpose(bigT_ap(big, c), sq)
            pairs = []
            for pi in range(2):
                pt = p_pair.tile([P, 2 * S], FP8, tag=f"{pair_tag}{pi}",
                                 name=f"{pair_tag}{pi}")
                for hf in range(2):
                    eng = nc.vector if (pi + hf) % 2 == 0 else nc.gpsimd
                    eng.tensor_copy(pt[:, hf * S:(hf + 1) * S],
                                    big[:, (pi * 2 + hf) * S:(pi * 2 + hf + 1) * S])
                pairs.append(pt)
            return seqs, big, pairs

        xb, xT, x8 = load_input(d["x"], "seqA", "bigA", "x8", 0)
        kqm_b = p_c.tile([P, NF * S], BF16, name="kqm_b")
        nc.sync.dma_start(
            out=kqm_b, in_=d["kqm"][:, :].rearrange("(a p) e -> p a e", p=P))
        kkm_b = p_c.tile([P, NF * S], BF16, name="kkm_b")
        nc.sync.dma_start(
            out=kkm_b, in_=d["kkm"][:, :].rearrange("(a p) e -> p a e", p=P))
        wq1t = load_wbf("wqc1"); wk1t = load_wbf("wkc1"); wv1t = load_wbf("wvc1")
        kp81 = load_w8("kp8_1", "kp8")
        ob, oT, o8 = load_input(d["obs"], "seqB", "bigB", "o8", 1)
        wg81 = load_w8("wg8_1", "wg8")
        wo81 = load_w8("wo8_1", "wo8")
        kksb = []
        for hc in range(2):
            kt = p_c.tile([P, 4 * E], BF16, name=f"kksb{hc}")
            nc.sync.dma_start(
                out=kt, in_=d["kks"][hc * 512:(hc + 1) * 512, :]
                .rearrange("(a p) e -> p a e", p=P))
            kksb.append(kt)
        wq2t = load_wbf("wqc2"); wk2t = load_wbf("wkc2"); wv2t = load_wbf("wvc2")
        kp82 = load_w8("kp8_2", "kp8"); wg82 = load_w8("wg8_2", "wg8")
        wo82 = load_w8("wo8_2", "wo8")
        fg8 = load_w8("fg8", "kp8"); fl8w = load_w8("fl8", "wg8")
        fo8 = load_w8("fo8", "wo8")

        def dump_seq(tiles, dt=F32):
            for c, t in enumerate(tiles[:NC]):
                o = p_ld.tile([P, E], F32, tag="dmp", name=f"dmp{c}")
                nc.vector.tensor_copy(o, t[:, :E])
                nc.sync.dma_start(out=out_h[c * P:(c + 1) * P, :], in_=o)

        def dump_fm(tiles, width=S):
            # feature-major tiles [128, width] -> out rows
            for k, t in enumerate(tiles[:NF]):
                o = p_ld.tile([P, S], F32, tag="dmpf", name=f"dmpf{k}", bufs=2)
                nc.vector.tensor_copy(o[:, :width], t[:, :width])
                rows = width // E
                for rr in range(rows):
                    nc.sync.dma_start(
                        out=out_h[(k * rows + rr) * P:(k * rows + rr + 1) * P, :],
                        in_=o[:, rr * E:(rr + 1) * E])

        # ================= MSR =================
        def msr(idx, qT_big, q8_pairs, kvT_big, kv8_pairs, resid_seq, wq_t, wk_t,
                wv_t, kp8, wg8, wo8, gn_triv, gsb, gbb, out_seq_tag, tap=""):
            sfx = str(idx)
            # --- q~ / k~ feature-major gemms (bf16) + decay-map evac ---
            qs = []
            ks = []
            for pt in range(NF):
                qs.append(p_act.tile([P, S], BF16, tag=f"qs{pt}", name=f"qs{sfx}_{pt}"))
                ks.append(p_act.tile([P, S], BF16, tag=f"ks{pt}", name=f"ks{sfx}_{pt}"))
            for pt in range(NF):
                for nh in range(2):
                    ps = pg.tile([P, 512], F32, tag="pg", name=f"q_{pt}_{nh}")
                    for k in range(NF):
                        nc.tensor.matmul(ps, wq_t[:, k * E + pt * P:
                                                   k * E + (pt + 1) * P],
                                         big_col(qT_big, k, nh * 512, 512),
                                         start=(k == 0), stop=(k == NF - 1))
                    nc.vector.tensor_mul(
                        qs[pt][:, nh * 512:(nh + 1) * 512], ps,
                        kqm_b[:, pt * S + nh * 512:pt * S + (nh + 1) * 512])
            for pt in range(NF):
                for nh in range(2):
                    ps = pg.tile([P, 512], F32, tag="pg", name=f"k_{pt}_{nh}")
                    for k in range(NF):
                        nc.tensor.matmul(ps, wk_t[:, k * E + pt * P:
                                                   k * E + (pt + 1) * P],
                                         big_col(kvT_big, k, nh * 512, 512),
                                         start=(k == 0), stop=(k == NF - 1))
                    nc.vector.tensor_mul(
                        ks[pt][:, nh * 512:(nh + 1) * 512], ps,
                        kkm_b[:, pt * S + nh * 512:pt * S + (nh + 1) * 512])
            if tap == "qs":
                dump_fm(qs)
                return None
            if tap == "ks":
                dump_fm(ks)
                return None

            # --- V (bf16) and K_seq (fp8 DR) seq-major gemms ---
            V = []
            Kq = []
            for c in range(NC):
                ps = pg.tile([P, 512], F32, tag="pg", name=f"v_{c}")
                for k in range(NF):
                    nc.tensor.matmul(ps, big_col(kvT_big, k, c * P, P),
                                     wv_t[:, k * E:(k + 1) * E],
                                     start=(k == 0), stop=(k == NF - 1))
                vt = p_kv.tile([P, 512], BF16, tag=f"V{c}", name=f"V{sfx}_{c}")
                nc.scalar.copy(vt, ps)
                V.append(vt)
            for c in range(NC):
                ps = pg.tile([P, 512], F32, tag="pg", name=f"kq_{c}")
                for pi in range(2):
                    nc.tensor.matmul(ps, pair_lhs(kv8_pairs[pi], pi, c),
                                     w8_rhs(kp8, pi), start=(pi == 0),
                                     stop=(pi == 1), perf_mode=DRM)
                kt = p_kv.tile([P, 512], BF16, tag=f"K{c}", name=f"K{sfx}_{c}")
                nc.vector.tensor_mul(
                    kt, ps, kksb[c // 4][:, (c % 4) * E:(c % 4 + 1) * E])
                Kq.append(kt)
            if tap == "V":
                dump_seq(V)
                return None
            if tap == "Kq":
                dump_seq(Kq)
                return None

            # --- gate gemm (fp8 DR, feature-major) + silu ---
            gfm = [p_act.tile([P, S], BF16, tag=f"gf{m}", name=f"gfm{sfx}_{m}")
                   for m in range(NF)]
            for m in ([] if "gate" in skips else range(NF)):
                for nh in range(2):
                    ps = pg.tile([P, 512], F32, tag="pg", name=f"g_{m}_{nh}")
                    for pi in range(2):
                        nc.tensor.matmul(ps, w8_lhs(wg8, pi, m),
                                         pair_rhs(q8_pairs[pi], nh),
                                         start=(pi == 0), stop=(pi == 1),
                                         perf_mode=DRM)
                    nc.scalar.activation(gfm[m][:, nh * 512:(nh + 1) * 512], ps,
                                         AF.Silu)

            # --- retention chunks (software-pipelined, batched GN stats) ---
            stc = []
            rn = []
            ret_sb = []
            sc_all = []
            sums_all = p_sm.tile([P, 64], F32, tag="sumsA", name=f"sumsA{sfx}",
                                 bufs=2)
            sqs_all = p_sm.tile([P, 64], F32, tag="sqsA", name=f"sqsA{sfx}",
                                bufs=2)

            def emit_scores(c):
                # bank parity = head parity so every matmul in a bank shares
                # the same tile_position row base (mixing row bases in one
                # bank breaks execution)
                sc_sb = []
                for par in range(2):
                    ps = psc.tile([P, 512], F32, tag="psc", name=f"sc_{c}_{par}")
                    sl = par * DH
                    for hh in range(4):
                        h = 2 * hh + par
                        ptt = h // 2
                        nc.tensor.matmul(
                            ps[:, hh * P:(hh + 1) * P],
                            ks[ptt][sl:sl + DH, c * P:(c + 1) * P],
                            qs[ptt][sl:sl + DH, c * P:(c + 1) * P],
                            start=(hh == 0), stop=(hh == 3),
                            skip_group_check=True)
                    sb = p_sc.tile([P, 512], BF16, tag=f"scsb{par}",
                                   name=f"scsb_{c}_{par}")
                    nc.vector.tensor_mul(sb, ps, cmask4)
                    sc_sb.append(sb)
                sc_all.append(sc_sb)

            def emit_stage(c):
                # state update first so chunk c+1's cross input is ready early
                if c < NC - 1 and "state" not in skips:
                    pstt = pst.tile([P, 256], F32, tag="pst", name=f"st_{c}",
                                    bufs=1)
                    for h in range(H):
                        ptt, sl = h // 2, (h % 2) * DH
                        nc.tensor.matmul(
                            pstt[sl:sl + DH, ptt * DH:(ptt + 1) * DH],
                            Kq[c][:, h * DH:(h + 1) * DH],
                            V[c][:, h * DH:(h + 1) * DH],
                            start=(h < 2), stop=(h >= H - 2),
                            skip_group_check=True, tile_position=(0, sl))
                    st = p_sm.tile([P, 256], BF16, tag="stc", name=f"stc_{c}",
                                   bufs=2)
                    if c == 0:
                        nc.vector.tensor_copy(st, pstt)
                    else:
                        nc.vector.tensor_add(st, pstt, stc[c - 1])
                    stc.append(st)
                prt = pret.tile([P, 512], F32, tag="pret", name=f"ret_{c}")
                cross = (c > 0) and ("state" not in skips) and \
                    ("intra" not in skips)
                for h in ([] if "intra" in skips else range(H)):
                    nc.tensor.matmul(
                        prt[:, h * DH:(h + 1) * DH],
                        sc_all[c][h % 2][:, (h // 2) * P:(h // 2 + 1) * P],
                        V[c][:, h * DH:(h + 1) * DH],
                        start=(h == 0), stop=(h == H - 1 and not cross),
                        skip_group_check=True)
                if cross:
                    for h in range(0, H, 2):       # even heads: row base 0
                        ptt = h // 2
                        nc.tensor.matmul(
                            prt[:, h * DH:(h + 1) * DH],
                            qs[ptt][0:DH, c * P:(c + 1) * P],
                            stc[c - 1][0:DH, ptt * DH:(ptt + 1) * DH],
                            start=False, stop=(h == H - 2),
                            skip_group_check=True)
                    pco = pcro.tile([P, 256], F32, tag="pcro", name=f"cro_{c}")
                    for h in range(1, H, 2):       # odd heads: row base 64
                        ptt = h // 2
                        nc.tensor.matmul(
                            pco[:, ptt * DH:(ptt + 1) * DH],
                            qs[ptt][DH:2 * DH, c * P:(c + 1) * P],
                            stc[c - 1][DH:2 * DH, ptt * DH:(ptt + 1) * DH],
                            start=(h == 1), stop=(h == H - 1),
                            skip_group_check=True)
                rb = p_seq.tile([P, 512], BF16, tag="retsb", name=f"retsb_{c}")
                if "intra" in skips:
                    nc.vector.memset(rb, 0.0)
                else:
                    nc.scalar.copy(rb, prt)
                    if cross:
                        def _odd(t, w):
                            return bass.AP(tensor=t.tensor,
                                           offset=t.offset + (DH if w else 0),
                                           ap=[list(t.ap[0]),
                                               [2 * DH if w else DH, 4],
                                               [1, DH]])
                        nc.vector.tensor_add(_odd(rb, True), _odd(rb, True),
                                             _odd(pco, False))
                ret_sb.append(rb)
                if "gn" not in skips:
                    sq = p_rot.tile([P, 512], BF16, tag="gnsq",
                                    name=f"gnsq_{c}")
                    nc.scalar.activation(sq, rb, AF.Square)
                    nc.vector.tensor_reduce(sums_all[:, c * H:(c + 1) * H],
                                            grp(rb),
                                            axis=mybir.AxisListType.X,
                                            op=ALU.add)
                    nc.vector.tensor_reduce(sqs_all[:, c * H:(c + 1) * H],
                                            grp(sq),
                                            axis=mybir.AxisListType.X,
                                            op=ALU.add)

            # narrow GN stats in two chunk-halves so the first half's
            # applies + rnT transposes overlap the second half's retention
            mu = p_sm.tile([P, 64], F32, tag="mu", name=f"mu{sfx}", bufs=1)
            m2 = p_sm.tile([P, 64], F32, tag="m2", name=f"m2{sfx}", bufs=1)
            msq = p_sm.tile([P, 64], F32, tag="msq", name=f"msq{sfx}", bufs=1)
            var = p_sm.tile([P, 64], F32, tag="var", name=f"var{sfx}", bufs=1)
            sd = p_sm.tile([P, 64], F32, tag="sd", name=f"sd{sfx}", bufs=1)
            rs = p_sm.tile([P, 64], F32, tag="rs", name=f"rs{sfx}", bufs=1)
            c2 = p_sm.tile([P, 64], F32, tag="c2", name=f"c2{sfx}", bufs=1)

            def bc8(t, c):
                return bass.AP(tensor=t.tensor, offset=t.offset + c * H,
                               ap=[list(t.ap[0]), [1, H], [0, DH]])

            def stats_and_apply(hf):
                hs = slice(hf * 32, hf * 32 + 32)
                nc.vector.tensor_scalar_mul(mu[:, hs], sums_all[:, hs],
                                            1.0 / DH)
                nc.gpsimd.tensor_mul(m2[:, hs], mu[:, hs], mu[:, hs])
                nc.gpsimd.tensor_scalar_mul(msq[:, hs], sqs_all[:, hs],
                                            1.0 / DH)
                nc.vector.tensor_sub(var[:, hs], msq[:, hs], m2[:, hs])
                nc.scalar.activation(sd[:, hs], var[:, hs], AF.Sqrt,
                                     bias=eps_gn)
                nc.vector.reciprocal(rs[:, hs], sd[:, hs])
                nc.gpsimd.tensor_mul(c2[:, hs], mu[:, hs], rs[:, hs])
                for c in range(hf * 4, hf * 4 + 4):
                    tmp = p_rot.tile([P, 512], BF16, tag="gntmp",
                                     name=f"gntmp_{c}")
                    nc.gpsimd.tensor_mul(grp(tmp), grp(ret_sb[c]), bc8(rs, c))
                    rt = p_rot.tile([P, 512], BF16, tag="rn", name=f"rn_{c}",
                                    bufs=2)
                    nc.gpsimd.tensor_sub(grp(rt), grp(tmp), bc8(c2, c))
                    if not gn_triv:
                        nc.vector.tensor_mul(rt, rt, gsb)
                        nc.vector.tensor_add(rt, rt, gbb)
                    rn.append(rt)
                    if tap != "rnnt":
                        qeng = nc.scalar if c % 2 else nc.sync
                        qeng.dma_start_transpose(bigT_ap(rnT, c), rt)

            import os as _os2
            if _os2.environ.get("KNOSPLIT"):
                emit_scores(0)
                for c in range(1, NC):
                    emit_scores(c)
                    emit_stage(c - 1)
                emit_stage(NC - 1)
                if tap == "ret":
                    dump_seq(ret_sb)
                    return None
                stats_and_apply(0)
                stats_and_apply(1)
            else:
                emit_scores(0)
                for c in range(1, NC):
                    emit_scores(c)
                    emit_stage(c - 1)
                    if c == 5:
                        stats_and_apply(0)
                emit_stage(NC - 1)
                if tap == "ret":
                    dump_seq(ret_sb)
                    return None
                stats_and_apply(1)
            if tap in ("rn", "rnnt"):
                dump_seq(rn)
                return None

            # --- gating (feature-major) -> fp8 pair tiles ---
            g8 = [p_pair.tile([P, 2 * S], FP8, tag=f"gated{pi}",
                              name=f"gated{sfx}_{pi}") for pi in range(2)]
            for m in range(NF):
                nc.gpsimd.tensor_mul(
                    g8[m // 2][:, (m % 2) * S:(m % 2 + 1) * S],
                    gfm[m], rnT[:, m * S:(m + 1) * S])

            # --- W_O gemm (fp8 DR, seq-major out) + residual + RMSNorm ---
            outs = []
            for c in range(NC):
                ps = pg.tile([P, 512], F32, tag="pg", name=f"wo_{c}")
                for pi in range(2):
                    nc.tensor.matmul(ps, pair_lhs(g8[pi], pi, c),
                                     w8_rhs(wo8, pi), start=(pi == 0),
                                     stop=(pi == 1), perf_mode=DRM)
                res = p_res.tile([P, E], F32, tag="res", name=f"res{sfx}_{c}")
                nc.vector.tensor_add(res, ps, resid_seq[c])
                ssq = p_sm.tile([P, 1], F32, tag="ssq", name=f"ssq_{c}", bufs=2)
                ts = p_rot.tile([P, E], BF16, tag="gnsq", name=f"ttr_{c}")
                nc.scalar.activation(ts, res, AF.Square)
                nc.vector.tensor_reduce(ssq, ts, axis=mybir.AxisListType.X,
                                        op=ALU.add)
                sdr = p_sm.tile([P, 1], F32, tag="sdr", name=f"sdr_{c}", bufs=2)
                nc.scalar.activation(sdr, ssq, AF.Sqrt, bias=eps_rms,
                                     scale=1.0 / E)
                rsr = p_sm.tile([P, 1], F32, tag="rsr", name=f"rsr_{c}", bufs=2)
                nc.vector.reciprocal(rsr, sdr)
                o = p_seq.tile([P, E], BF16, tag=out_seq_tag,
                               name=f"{out_seq_tag}{c}")
                nc.scalar.activation(o, res, AF.Identity, scale=rsr)
                outs.append(o)
            return outs

        # rnT big tile shared by both msrs (rotates)
        rnT = p_big.tile([P, NF * S], BF16, tag="bigR", name="rnT1")
        r = msr(1, xT, x8, xT, x8, xb, wq1t, wk1t,
                wv1t, kp81, wg81, wo81,
                gn1_triv, gcons.get("gsb1"), gcons.get("gbb1"), "seqA",
                tap=tap if tap in ("qs", "ks", "V", "Kq", "rn", "ret", "rnnt") else "")
        if tap in ("qs", "ks", "V", "Kq", "rn", "ret", "rnnt"):
            return
        x1 = r
        if tap == "x1":
            dump_seq(x1)
            return

        # x1 -> feature-major (dma transpose) + fp8 pairs
        x1T = p_big.tile([P, NF * S], BF16, tag="bigA", name="x1T")
        for c in range(NC):
            qeng = nc.scalar if c % 2 else nc.sync
            qeng.dma_start_transpose(bigT_ap(x1T, c), x1[c])
        x18 = []
        for pi in range(2):
            pt = p_pair.tile([P, 2 * S], FP8, tag=f"x8{pi}", name=f"x18_{pi}")
            for hf in range(2):
                eng = nc.vector if (pi + hf) % 2 == 0 else nc.gpsimd
                eng.tensor_copy(pt[:, hf * S:(hf + 1) * S],
                                x1T[:, (pi * 2 + hf) * S:(pi * 2 + hf + 1) * S])
            x18.append(pt)

        rnT = p_big.tile([P, NF * S], BF16, tag="bigR", name="rnT2")
        r = msr(2, oT, o8, x1T, x18, ob, wq2t, wk2t,
                wv2t, kp82, wg82, wo82,
                gn2_triv, gcons.get("gsb2"), gcons.get("gbb2"), "seqC")
        x2 = r
        if not ln2_triv:
            for c in range(NC):
                nc.gpsimd.tensor_mul(x2[c], x2[c], gcons["ln2C"])
        if tap == "x2":
            dump_seq(x2)
            return

        # x2 -> feature-major + fp8 pairs (ffn inputs)
        x2T = p_big.tile([P, NF * S], BF16, tag="bigR", name="x2T")
        for c in range(NC):
            qeng = nc.scalar if c % 2 else nc.sync
            qeng.dma_start_transpose(bigT_ap(x2T, c), x2[c])
        x28 = []
        for pi in range(2):
            pt = p_pair.tile([P, 2 * S], FP8, tag=f"o8{pi}", name=f"x28_{pi}")
            for hf in range(2):
                eng = nc.vector if (pi + hf) % 2 == 0 else nc.gpsimd
                eng.tensor_copy(pt[:, hf * S:(hf + 1) * S],
                                x2T[:, (pi * 2 + hf) * S:(pi * 2 + hf + 1) * S])
            x28.append(pt)

        # ---- FFN (all fp8 DR) ----
        ffg = [p_act.tile([P, S], BF16, tag=f"qs{m}", name=f"ffg_{m}")
               for m in range(NF)]
        ffl = [p_act.tile([P, S], BF16, tag=f"ks{m}", name=f"ffl_{m}")
               for m in range(NF)]
        for m in range(NF):
            for nh in range(2):
                ps = pg.tile([P, 512], F32, tag="pg", name=f"fg_{m}_{nh}")
                for pi in range(2):
                    nc.tensor.matmul(ps, w8_lhs(fg8, pi, m),
                                     pair_rhs(x28[pi], nh), start=(pi == 0),
                                     stop=(pi == 1), perf_mode=DRM)
                nc.scalar.activation(ffg[m][:, nh * 512:(nh + 1) * 512], ps,
                                     AF.Silu)
        for m in range(NF):
            for nh in range(2):
                ps = pg.tile([P, 512], F32, tag="pg", name=f"fl_{m}_{nh}")
                for pi in range(2):
                    nc.tensor.matmul(ps, w8_lhs(fl8w, pi, m),
                                     pair_rhs(x28[pi], nh), start=(pi == 0),
                                     stop=(pi == 1), perf_mode=DRM)
                nc.vector.tensor_copy(ffl[m][:, nh * 512:(nh + 1) * 512], ps)
        fl8t = [p_pair.tile([P, 2 * S], FP8, tag=f"gated{pi}", name=f"flT8_{pi}")
                for pi in range(2)]
        for m in range(NF):
            nc.gpsimd.tensor_mul(fl8t[m // 2][:, (m % 2) * S:(m % 2 + 1) * S],
                                 ffg[m], ffl[m])
        for c in range(NC):
            ps = pg.tile([P, 512], F32, tag="pg", name=f"fo_{c}")
            for pi in range(2):
                nc.tensor.matmul(ps, pair_lhs(fl8t[pi], pi, c),
                                 w8_rhs(fo8, pi), start=(pi == 0),
                                 stop=(pi == 1), perf_mode=DRM)
            res = p_res.tile([P, E], F32, tag="res", name=f"res3_{c}")
            nc.vector.tensor_add(res, ps, x2[c])
            ssq = p_sm.tile([P, 1], F32, tag="ssq", name=f"ssq3_{c}", bufs=2)
            ts = p_rot.tile([P, E], BF16, tag="gnsq", name=f"ttr3_{c}")
            nc.scalar.activation(ts, res, AF.Square)
            nc.vector.tensor_reduce(ssq, ts, axis=mybir.AxisListType.X,
                                    op=ALU.add)
            sdr = p_sm.tile([P, 1], F32, tag="sdr", name=f"sdr3_{c}", bufs=2)
            nc.scalar.activation(sdr, ssq, AF.Sqrt, bias=eps_rms, scale=1.0 / E)
            rsr = p_sm.tile([P, 1], F32, tag="rsr", name=f"rsr3_{c}", bufs=2)
            nc.vector.reciprocal(rsr, sdr)
            if c % 4 == 0:
                obig = p_ld.tile([P, 4 * E], F32, tag="oo", name=f"oo_{c // 4}",
                                 bufs=1)
            o = obig[:, (c % 4) * E:(c % 4 + 1) * E]
            nc.scalar.activation(o, res, AF.Identity, scale=rsr)
            if not ln3_triv:
                nc.gpsimd.tensor_mul(o, o, gcons["ln3C"])
            if c % 4 == 3:
                nc.sync.dma_start(
                    out=out_h[(c - 3) * P:(c + 1) * P, :]
                    .rearrange("(a p) e -> p a e", p=P), in_=obig)


_prog_cache = {}


def _get_program(flags=(True, True, True, True)):
    if flags not in _prog_cache:
        _prog_cache[flags] = _build_program(flags)
    return _prog_cache[flags]


def kernel(**inputs):
    inputs = {k: np.asarray(v) for k, v in inputs.items()}
    flags = _flags(inputs)
    consts = _build_consts(inputs)
    pr = _get_program(flags)
    x = np.ascontiguousarray(inputs["x"], dtype=np.float32)
    obs = np.ascontiguousarray(inputs["obs_rep"], dtype=np.float32)
    in_maps = []
    for b in range(N_CORES):
        m = dict(consts)
        m["x"] = np.ascontiguousarray(x[b])
        m["obs"] = np.ascontiguousarray(obs[b])
        in_maps.append(m)
    res = run_bass_kernel_spmd(pr.nc, in_maps, core_ids=list(range(N_CORES)))
    return np.stack([res.results[b]["out"] for b in range(N_CORES)], axis=0)
